# revision 30
# baseline (speedup 1.0000x reference)
"""Trainium2 Bass kernel for the LeViT-style attention block.

Contract: kernel(**inputs) takes the FULL unsharded inputs (numpy) and
returns the FULL [128, 196, 576] float32 output. Internally shards the
batch dim across 8 NeuronCores (16 batches per core) and runs a single
SPMD Bass/Tile program via run_bass_kernel_spmd.

Math (per batch b):
  xn   = LayerNorm(x[b]) * g + beta                     [196, 576]
  qkv  = xn @ qkv_w.T + qkv_b      -> q,k,v per head
  S_h  = (q_h * kd^-0.5) @ k_h.T + bias_h               [196, 196]
  P_h  = softmax(S_h, axis=-1)
  O_h  = P_h @ v_h                                      [196, 128]
  out  = concat_h(O_h) @ proj_w.T + proj_b              [196, 576]

v3 implementation notes (cost-model driven):
  - matmul cost = out-free-size x cycles/row of the MOVING operand
    (fp16/bf16 = 1 always; f32r = 1 only when free >= 256; fp32 = 4);
    contraction depth, partition fill and stationary loads are free, so
    everything 2-byte-able is fp16 (precision) or bf16 (range).
  - token chunks are the OVERLAPPING pairs (0,128) and (68,128): every
    PE output keeps a full, base-0 partition range.  The 60-key overlap
    in the second chunk is killed with -60000 rows in the bias seed
    (exp -> exact 0), so denominator/PV stay correct.
  - scores: per-head bias seeded into PSUM by an identity matmul (fp16,
    196/row), score matmul accumulates on top; exp on ACT; denominator
    via ones-column matmul on PE (PSUM-accumulated across key chunks);
    reciprocal on DVE; ones-row broadcast matmul on PE; normalize fused
    into the PSUM->SBUF O.T copy on DVE.
  - per-head O.T resident in SBUF fp16 (no DRAM scratch); proj streams
    it with fp16 moving operands into 5 resident PSUM banks.
  - weights/bias constants ship as fp16 in a handful of large DMAs on
    the SP (HWDGE) queue, after the first x loads, ordered by first use
    (the Pool/SWDGE queue's per-DMA descriptor-gen cost would starve
    the first superbatches).
  - PSUM->SBUF copies are spread over Pool/ACT/DVE to keep every
    engine under the PE budget.
"""

import os

os.environ.setdefault("MYCRO_LOCAL_CACHE", "1")

from contextlib import ExitStack

import numpy as np
import ml_dtypes

import concourse.bass as bass
import concourse.mybir as mybir
import concourse.tile as tile
from concourse import masks
from concourse.bass import AP
from concourse.bass_utils import run_bass_kernel_spmd

# Problem shape (hardcoded per contest contract).
B, N, C = 128, 196, 576
H, KD, DV = 18, 32, 128
DH = H * DV            # 2304
LN_EPS = 1e-5
SCALE = KD ** -0.5
NCORES = 8
BPC = B // NCORES      # 16 batches per core
SB = 2                 # batches per "superbatch" iteration
NSB = BPC // SB        # 8
W = SB * N             # 392: packed two-batch free dim

FP32 = mybir.dt.float32
F32R = mybir.dt.float32r
FP16 = mybir.dt.float16
BF16 = mybir.dt.bfloat16

# token-dim chunks: overlapping full-128 chunks
TOK_CHUNKS = [(0, 128), (68, 128)]
# C-dim chunks (576 = 4*128 + 64)
C_CHUNKS = [(i * 128, min(128, C - i * 128)) for i in range((C + 127) // 128)]
NCC = len(C_CHUNKS)
# V free-dim chunks of 512 = 4 heads
V_CHUNKS = [(i * 512, min(512, DH - i * 512)) for i in range((DH + 511) // 512)]


def _split_multiwaits(nc):
    """This container's walrus rejects >1 sync-wait per instruction
    (TPB EVENTS struct has a single wait slot). Split extras into
    preceding same-engine NOPs — semantically identical."""
    for f in nc.m.functions:
        for blk in f.blocks:
            newlist = []
            changed = False
            for inst in blk.instructions:
                si = inst.sync_info
                waits = list(si.on_wait) if si is not None else []
                if len(waits) > 1:
                    changed = True
                    for j, w in enumerate(waits[:-1]):
                        nop = mybir.InstNoOp(name=f"{inst.name}_sw{j}", ins=[], outs=[])
                        nop.engine = inst.engine
                        nop.sync_info = mybir.SyncInfo(on_wait=[w], on_update=[])
                        newlist.append(nop)
                    inst.sync_info = mybir.SyncInfo(
                        on_wait=[waits[-1]], on_update=list(si.on_update)
                    )
                newlist.append(inst)
            if changed:
                blk.instructions = newlist


# 10 QK output groups over the packed wqkt columns
# [Q 0:512 | Q 512:576 | K 512:576 | K 0:512]: 4x128 Q, 64 Q-tail,
# 64 K-tail, 4x128 K. Within every group a head's 32 rows sit at the
# same offset for q and k (the PE pairs moving/stationary rows by
# absolute partition, so offsets must match).
QK_GROUPS = [(128 * j, 128) for j in range(4)] + [(512, 64), (576, 64)] + [
    (640 + 128 * j, 128) for j in range(4)
]


def _qk_slice(h):
    """(qgroup, kgroup, offset) for head h."""
    if h < 16:
        return h // 4, 6 + h // 4, (32 * h) % 128
    return 4, 5, 32 * (h - 16)


def _emit(ctx: ExitStack, tc: tile.TileContext, aps: dict, has_vbias: bool):
    nc = tc.nc
    x_d = aps["x"]          # [BPC, 196, 576] f32
    out_d = aps["out"]      # [BPC, 196, 576] f32
    wqk_d = aps["wqkt"]     # [576, 1152] fp16, packed as
                            # [Wq.T rows0:512 | Wq.T 512:576 | Wk.T 512:576 |
                            #  Wk.T rows0:512] so all 9 groups are contiguous
    wv_d = aps["wvt"]       # [576, 2304] fp16 (Wv.T, head-major columns)
    pw_d = aps["pwp"]       # [128, 18*576] fp16 (proj_w.T packed per head)
    bias_d = aps["biasp"]   # [128, 18*392] fp16 (seed bias, chunked
                            #  [head, key-chunk, query]; overlap rows -60000)
    qkb_d = aps["qkb"]      # [128, 9] f32 (per-group qk bias columns)
    pb_d = aps["pb"]        # [128, 5] f32 (proj bias chunks)
    vb_d = aps.get("vb")    # [128, 2304] f32 (replicated v bias) — optional

    cpool = ctx.enter_context(tc.tile_pool(name="consts", bufs=1))
    xpool = ctx.enter_context(tc.tile_pool(name="x", bufs=2))
    xnpool = ctx.enter_context(tc.tile_pool(name="xn", bufs=2))
    stat = ctx.enter_context(tc.tile_pool(name="stat", bufs=2))
    sqpool = ctx.enter_context(tc.tile_pool(name="sq", bufs=2))
    xnt_pool = ctx.enter_context(tc.tile_pool(name="xnt", bufs=1))
    qkt_pool = ctx.enter_context(tc.tile_pool(name="qkt", bufs=1))
    vpool = ctx.enter_context(tc.tile_pool(name="v", bufs=2))
    epool = ctx.enter_context(tc.tile_pool(name="e", bufs=3))
    dnpool = ctx.enter_context(tc.tile_pool(name="dn", bufs=2))
    dspool = ctx.enter_context(tc.tile_pool(name="ds", bufs=2))
    rcpool = ctx.enter_context(tc.tile_pool(name="rc", bufs=3))
    rbpool = ctx.enter_context(tc.tile_pool(name="rb", bufs=3))
    oupool = ctx.enter_context(tc.tile_pool(name="ou", bufs=3))
    onpool = ctx.enter_context(tc.tile_pool(name="on", bufs=1))
    ftpool = ctx.enter_context(tc.tile_pool(name="ft", bufs=1))
    fpool = ctx.enter_context(tc.tile_pool(name="f", bufs=2))
    ps = ctx.enter_context(tc.tile_pool(name="ps", bufs=8, space="PSUM"))

    # ---- small on-chip constants ----
    ident = cpool.tile([128, 128], FP16, tag="ident", name="ident")
    masks.make_identity(nc, ident[:])
    epsb = cpool.tile([128, 1], FP32, tag="epsb", name="epsb")
    nc.gpsimd.memset(epsb[:], LN_EPS)

    # ---- resident weight tiles (DMAs issued after the first x loads so
    # the SP queue serves x first; ordered by first use) ----
    wqk = [cpool.tile([128, 2 * C], FP16, tag=f"wqk{ci}", name=f"wqk{ci}")
           for ci in range(NCC)]
    wv = [cpool.tile([128, DH], FP16, tag=f"wv{ci}", name=f"wv{ci}")
          for ci in range(NCC)]
    pw = cpool.tile([128, H * C], FP16, tag="pw", name="pw")
    biasT = cpool.tile([128, H * W], FP16, tag="biasT", name="biasT")
    qkb = cpool.tile([128, 10], FP32, tag="qkb", name="qkb")
    pb = cpool.tile([128, NCC], FP32, tag="pb", name="pb")
    vb = None
    if has_vbias:
        vb = cpool.tile([128, DH], FP32, tag="vb", name="vb")

    def load_weights():
        for ci, (c0, cs) in enumerate(C_CHUNKS):
            nc.sync.dma_start(wqk[ci][:cs, :], wqk_d[c0 : c0 + cs, :])
        nc.sync.dma_start(qkb[:], qkb_d[:])
        nc.sync.dma_start(biasT[:], bias_d[:])
        for ci, (c0, cs) in enumerate(C_CHUNKS):
            nc.sync.dma_start(wv[ci][:cs, :], wv_d[c0 : c0 + cs, :])
        if has_vbias:
            nc.sync.dma_start(vb[:], vb_d[:])
        nc.sync.dma_start(pw[:], pw_d[:])
        nc.sync.dma_start(pb[:], pb_d[:])

    inv_c = 1.0 / C

    lnstate = {}
    lnmid = {}
    xstate = {}

    def ln_load(sbx, q, tj):
        b = sbx * SB + q
        t0, ts_ = TOK_CHUNKS[tj]
        xt = xpool.tile([128, C], FP32, tag=f"xb{q}{tj}", name=f"xb{q}{tj}_{sbx}")
        nc.sync.dma_start(xt[:ts_, :], x_d[b, t0 : t0 + ts_, :])
        xstate[(sbx, q, tj)] = xt

    def ln_chunk(sbx, q, tj):
        ln_stats(sbx, q, tj)
        ln_apply(sbx, q, tj)

    def ln_stats(sbx, q, tj):
        """LayerNorm stats for one loaded (batch, token-chunk)."""
        t0, ts_ = TOK_CHUNKS[tj]
        xt = xstate.pop((sbx, q, tj))
        negmu = stat.tile([128, 1], FP32, tag="negmu", name=f"nm{q}{tj}_{sbx}")
        nc.vector.tensor_reduce(
            negmu[:ts_], xt[:ts_, :], axis=mybir.AxisListType.X,
            op=mybir.AluOpType.add, negate=True,
        )
        nc.vector.tensor_scalar_mul(negmu[:ts_], negmu[:ts_], inv_c)
        sq = sqpool.tile([128, C], FP32, tag="sq", name=f"sq{q}{tj}_{sbx}")
        ha = stat.tile([128, 1], FP32, tag="ha", name=f"ha{q}{tj}_{sbx}")
        nc.scalar.activation(
            sq[:ts_, :], xt[:ts_, :],
            mybir.ActivationFunctionType.Square, bias=negmu[:ts_], accum_out=ha[:ts_],
        )
        std = stat.tile([128, 1], FP32, tag="std", name=f"std{q}{tj}_{sbx}")
        nc.scalar.activation(
            std[:ts_], ha[:ts_], mybir.ActivationFunctionType.Sqrt,
            bias=epsb[:ts_], scale=inv_c,
        )
        r = stat.tile([128, 1], FP32, tag="r", name=f"r{q}{tj}_{sbx}")
        nc.vector.reciprocal(r[:ts_], std[:ts_])
        negmur = stat.tile([128, 1], FP32, tag="negmur", name=f"nr{q}{tj}_{sbx}")
        nc.vector.tensor_mul(negmur[:ts_], negmu[:ts_], r[:ts_])
        lnmid[(sbx, q, tj)] = (xt, negmur, r)

    def ln_apply(sbx, q, tj):
        """normalize in fp16 from the precomputed stats."""
        t0, ts_ = TOK_CHUNKS[tj]
        xt, negmur, r = lnmid.pop((sbx, q, tj))
        xn = xnpool.tile([128, C], FP16, tag=f"xn{q}{tj}", name=f"xn{q}{tj}_{sbx}")
        nc.vector.tensor_scalar(
            xn[:ts_, :], xt[:ts_, :], r[:ts_], negmur[:ts_],
            op0=mybir.AluOpType.mult, op1=mybir.AluOpType.add,
        )
        lnstate[(sbx, q, tj)] = xn

    pending_finals = []

    for q in range(SB):
        for tj in range(len(TOK_CHUNKS)):
            ln_load(0, q, tj)
    load_weights()
    for q in range(SB):
        for tj in range(len(TOK_CHUNKS)):
            ln_chunk(0, q, tj)

    xnt_staged = {}

    def emit_transpose(sbx, q, tj, xnt):
        t0, ts_ = TOK_CHUNKS[tj]
        xn = lnstate.pop((sbx, q, tj))
        for ci, (c0, cs) in enumerate(C_CHUNKS):
            pt = ps.tile([128, 512], FP16, tag="ps", name=f"pst{q}{tj}{ci}_{sbx}")
            nc.tensor.transpose(
                pt[:cs, :ts_], xn[:ts_, c0 : c0 + cs], ident[:ts_, :ts_]
            )
            col = q * N + t0
            if ci % 2 == 0:
                nc.scalar.copy(xnt[ci][:cs, col : col + ts_], pt[:cs, :ts_])
            else:
                nc.vector.tensor_copy(xnt[ci][:cs, col : col + ts_], pt[:cs, :ts_])

    for sb in range(NSB):
        # ---- xn -> xnT (channel-major): staged during the previous
        # superbatch's proj pass, except for the first superbatch ----
        if sb in xnt_staged:
            xnt = xnt_staged.pop(sb)
        else:
            xnt = [
                xnt_pool.tile([128, W], FP16, tag=f"xnt{ci}", name=f"xnt{ci}_{sb}")
                for ci in range(NCC)
            ]
            for q in range(SB):
                for tj in range(len(TOK_CHUNKS)):
                    emit_transpose(sb, q, tj, xnt)

        if sb + 1 < NSB:
            for q in range(SB):
                for tj in range(len(TOK_CHUNKS)):
                    ln_load(sb + 1, q, tj)

        # ---- Q.T / K.T GEMMs (9 groups of 128 rows, both batches) ----
        qkt = [None] * 10
        for n_, j in enumerate((0, 6, 1, 7, 2, 8, 3, 9, 4, 5)):
            col0, rows = QK_GROUPS[j]
            pq = ps.tile([128, 512], FP32, tag="ps", name=f"psqk{j}_{sb}")
            for ci, (c0, cs) in enumerate(C_CHUNKS):
                nc.tensor.matmul(
                    pq[:rows, :W], wqk[ci][:cs, col0 : col0 + rows],
                    xnt[ci][:cs, :W],
                    start=(ci == 0), stop=(ci == NCC - 1),
                )
            t = qkt_pool.tile([128, W], FP16, tag=f"qkt{j}", name=f"qkt{j}_{sb}")
            if n_ % 2 == 0:
                nc.vector.tensor_scalar_add(
                    t[:rows, :], pq[:rows, :W], qkb[:rows, j : j + 1]
                )
            else:
                nc.scalar.activation(
                    t[:rows, :], pq[:rows, :W],
                    mybir.ActivationFunctionType.Identity, bias=qkb[:rows, j : j + 1],
                )
            qkt[j] = t

        # ---- attention: software-pipelined head loop ----
        vgroups = {}
        estate = {}
        b1state = {}
        onorm = [
            onpool.tile([128, W], FP16, tag=f"on{h}", name=f"on{h}_{sb}")
            for h in range(H)
        ]

        def emit_v_chunk(g, k, sb=sb, xnt=xnt, vgroups=vgroups):
            n0, ns = V_CHUNKS[g]
            q, tj = k // 2, k % 2
            t0, ts_ = TOK_CHUNKS[tj]
            v = vpool.tile([128, 512], BF16, tag=f"v{q}{tj}",
                           name=f"v{q}{tj}g{g}_{sb}")
            pv = ps.tile([128, 512], FP32, tag="ps", name=f"psv{q}{tj}{g}_{sb}")
            for ci, (c0, cs) in enumerate(C_CHUNKS):
                nc.tensor.matmul(
                    pv[:ts_, :ns], xnt[ci][:cs, q * N + t0 : q * N + t0 + ts_],
                    wv[ci][:cs, n0 : n0 + ns],
                    start=(ci == 0), stop=(ci == NCC - 1),
                )
            if has_vbias:
                nc.vector.tensor_add(
                    v[:ts_, :ns], pv[:ts_, :ns], vb[:ts_, n0 : n0 + ns]
                )
            elif (q + tj) % 2 == 0:
                nc.scalar.copy(v[:ts_, :ns], pv[:ts_, :ns])
            else:
                nc.vector.tensor_copy(v[:ts_, :ns], pv[:ts_, :ns])
            vgroups.setdefault(g, {})[(q, tj)] = v

        def stage_a(h, sb=sb, qkt=qkt, estate=estate):
            # seeds + scores + exp for head h; S.T in [key, query] layout
            qg, kg, ro = _qk_slice(h)
            qt, kt = qkt[qg], qkt[kg]
            st = []
            for q in range(SB):
                s = ps.tile([128, 512], FP32, tag="ps", name=f"st{q}h{h}_{sb}")
                for tj, (t0, ts_) in enumerate(TOK_CHUNKS):
                    nc.tensor.matmul(
                        s[:128, tj * N : tj * N + N],
                        ident[:128, :],
                        biasT[:, (h * 2 + tj) * N : (h * 2 + tj) * N + N],
                        start=True, stop=False, skip_group_check=True,
                    )
                    nc.tensor.matmul(
                        s[:128, tj * N : tj * N + N],
                        kt[ro : ro + 32, q * N + t0 : q * N + t0 + ts_],
                        qt[ro : ro + 32, q * N : q * N + N],
                        start=False, stop=True, skip_group_check=True,
                        tile_position=(ro, 0),
                    )
                st.append(s)
            # e layout: [key-chunk, batch, query] so each key-chunk's
            # 392 columns are contiguous (dn needs a 1-D moving AP)
            e = epool.tile([128, 2 * W], BF16, tag="e", name=f"e{h}_{sb}")
            e4 = e[:, :].rearrange("p (c q n) -> p c q n", c=2, q=2)
            for q in range(SB):
                st2 = st[q][:, 0 : 2 * N].rearrange("p (c n) -> p c n", c=2)
                nc.scalar.activation(e4[:, :, q], st2,
                                     mybir.ActivationFunctionType.Exp)
            estate[h] = e

        def stage_b1(h, sb=sb, estate=estate, b1state=b1state):
            # denominator off the PE: partition-reduce on the idle Pool
            # engine, reciprocal on DVE, partition-broadcast via a
            # repeat-read DMA (no PE ones-matmuls at all)
            e = estate[h]
            dnp = dnpool.tile([1, 2 * W], FP32, tag="dn", name=f"dn{h}_{sb}")
            nc.gpsimd.tensor_reduce(
                dnp[:, :], e[:, :], axis=mybir.AxisListType.C,
                op=mybir.AluOpType.add,
            )
            ds = dspool.tile([1, W], FP32, tag="ds", name=f"ds{h}_{sb}")
            nc.gpsimd.tensor_add(ds[:, :], dnp[:, 0:W], dnp[:, W : 2 * W])
            rc = rcpool.tile([1, W], FP32, tag="rc", name=f"rc{h}_{sb}")
            nc.vector.reciprocal(rc[:, :], ds[:, :W])
            rb = rbpool.tile([128, W], FP32, tag="rb", name=f"rb{h}_{sb}")
            s_ap = rc[0:1, :]
            nc.sync.dma_start(
                rb[:, :], AP(s_ap.tensor, s_ap.offset, [[W, 1], [0, 128], [1, W]])
            )
            b1state[h] = rb

        def stage_pv(h, sb=sb, estate=estate, vgroups=vgroups):
            e = estate.pop(h)
            g = h // 4
            vt = vgroups[g]
            n0, ns = V_CHUNKS[g]
            hcol = h * DV - n0
            ou = ps.tile([128, 512], FP32, tag="ps", name=f"ou{h}_{sb}")
            for q in range(SB):
                for tj, (t0, ts_) in enumerate(TOK_CHUNKS):
                    nc.tensor.matmul(
                        ou[:DV, q * N : q * N + N],
                        vt[(q, tj)][:ts_, hcol : hcol + DV],
                        e[:, tj * W + q * N : tj * W + q * N + N],
                        start=(tj == 0), stop=(tj == 1),
                    )
            # stage the unnormalized O.T to SBUF right away (bf16 for
            # range: values can exceed fp16 max) so the PSUM bank frees
            # without waiting for the broadcast reciprocal
            ousb = oupool.tile([128, W], BF16, tag="ou", name=f"ousb{h}_{sb}")
            nc.scalar.copy(ousb[:DV, :], ou[:DV, :W])
            return ousb

        def stage_b3(h, ousb, sb=sb, b1state=b1state, onorm=onorm):
            # normalize O.T into its resident fp16 slot (DVE, all-SBUF)
            rb = b1state.pop(h)
            nc.vector.tensor_mul(onorm[h][:, :], ousb[:DV, :], rb[:, :])

        # pipeline: dn/recip lead by 1 head, broadcast trails so the PE
        # queue never waits on the DVE reciprocal
        for k in range(4):
            emit_v_chunk(0, k)
        stage_a(0)
        stage_a(1)
        stage_b1(0)
        oustate = {}
        for h in range(H):
            g_next = h // 4 + 1
            if g_next <= 4:
                emit_v_chunk(g_next, h % 4)
            if h + 2 < H:
                stage_a(h + 2)
            if h + 1 < H:
                stage_b1(h + 1)
            oustate[h] = stage_pv(h)
            if h > 0:
                stage_b3(h - 1, oustate.pop(h - 1))
            if pending_finals and h in (0, 1, 2, 4):
                pending_finals.pop(0)()
            if sb + 1 < NSB:
                if h in (5, 7, 9, 11):
                    k = (5, 7, 9, 11).index(h)
                    ln_stats(sb + 1, k // 2, k % 2)
                elif h in (6, 8, 10, 12):
                    k = (6, 8, 10, 12).index(h)
                    ln_apply(sb + 1, k // 2, k % 2)

        stage_b3(H - 1, oustate.pop(H - 1))

        # ---- proj: accumulate heads from resident O.T, with the next
        # superbatch's xn transposes interleaved (PE stays busy on proj
        # while the transpose copies drain on ACT/DVE) ----
        ft = ftpool.tile([128, NCC * W], FP16, tag="ft", name=f"ft{sb}")
        if sb + 1 < NSB:
            xnt_next = [
                xnt_pool.tile([128, W], FP16, tag=f"xnt{ci}", name=f"xnt{ci}_{sb + 1}")
                for ci in range(NCC)
            ]
            xnt_staged[sb + 1] = xnt_next
        # m-outer / h-inner: one PSUM bank at a time (onorm re-reads from
        # SBUF are free), leaving banks for the interleaved transposes
        for m in range(NCC):
            c0, mc = C_CHUNKS[m]
            if sb + 1 < NSB and m < 4:
                emit_transpose(sb + 1, m // 2, m % 2, xnt_next)
            pp = ps.tile([128, W], FP32, tag="ps", name=f"pp{m}_{sb}")
            for h in range(H):
                nc.tensor.matmul(
                    pp[:mc, :W], pw[:, h * C + c0 : h * C + c0 + mc],
                    onorm[h][:DV, :W],
                    start=(h == 0), stop=(h == H - 1),
                )
            nc.vector.tensor_scalar_add(
                ft[:mc, m * W : m * W + W], pp[:mc, :W], pb[:mc, m : m + 1]
            )

        # ---- transpose back to token layout and store (deferred into the
        # next superbatch's head loop; flushed immediately on the last) ----
        def make_final(q, tj, ft=ft, sb=sb):
            def emit():
                b = sb * SB + q
                t0, ts_ = TOK_CHUNKS[tj]
                f = fpool.tile([128, C], FP32, tag="f", name=f"f{q}{tj}_{sb}")
                for m, (c0, mc) in enumerate(C_CHUNKS):
                    pt = ps.tile([128, 512], FP16, tag="ps", name=f"psf{m}{q}{tj}_{sb}")
                    src0 = m * W + q * N + t0
                    nc.tensor.transpose(
                        pt[:ts_, :mc], ft[:mc, src0 : src0 + ts_], ident[:mc, :mc]
                    )
                    if m % 2 == 0:
                        nc.scalar.copy(f[:ts_, c0 : c0 + mc], pt[:ts_, :mc])
                    else:
                        nc.vector.tensor_copy(f[:ts_, c0 : c0 + mc], pt[:ts_, :mc])
                nc.sync.dma_start(out_d[b, t0 : t0 + ts_, :], f[:ts_, :])
            return emit

        chunks = [make_final(q, tj) for q in range(SB) for tj in range(2)]
        if sb + 1 < NSB:
            pending_finals.extend(chunks)
        else:
            for c in chunks:
                c()


def _build(has_vbias: bool):
    nc = bass.Bass(
        "TRN2", target_bir_lowering=False, debug=False,
        enable_asserts=False, num_devices=NCORES,
    )
    aps = {}
    aps["x"] = nc.dram_tensor("x", [BPC, N, C], FP32, kind="ExternalInput").ap()
    aps["wqkt"] = nc.dram_tensor("wqkt", [C, 2 * C], FP16, kind="ExternalInput").ap()
    aps["wvt"] = nc.dram_tensor("wvt", [C, DH], FP16, kind="ExternalInput").ap()
    aps["pwp"] = nc.dram_tensor("pwp", [128, H * C], FP16, kind="ExternalInput").ap()
    aps["biasp"] = nc.dram_tensor("biasp", [128, H * W], FP16, kind="ExternalInput").ap()
    aps["qkb"] = nc.dram_tensor("qkb", [128, 10], FP32, kind="ExternalInput").ap()
    aps["pb"] = nc.dram_tensor("pb", [128, NCC], FP32, kind="ExternalInput").ap()
    if has_vbias:
        aps["vb"] = nc.dram_tensor("vb", [128, DH], FP32, kind="ExternalInput").ap()
    aps["out"] = nc.dram_tensor("out", [BPC, N, C], FP32, kind="ExternalOutput").ap()

    with tile.TileContext(nc) as tc, ExitStack() as ctx:
        with nc.allow_low_precision(reason="fp16/bf16 matmul pipeline"):
            _emit(ctx, tc, aps, has_vbias)
    _split_multiwaits(nc)
    return nc


_BUILD_CACHE: dict = {}


def _prep_host(x, ln_g, ln_b, qkv_w, qkv_b, proj_w, proj_b, attn_biases, bias_idxs):
    """Permute/fold weights host-side. Returns (in_map_consts, has_vbias)."""
    f32 = np.float32
    f16 = np.float16
    qkv_w = np.asarray(qkv_w, f32)
    qkv_b = np.asarray(qkv_b, f32)
    ln_g = np.asarray(ln_g, f32)
    ln_b = np.asarray(ln_b, f32)
    proj_w = np.asarray(proj_w, f32)
    proj_b = np.asarray(proj_b, f32)
    attn_biases = np.asarray(attn_biases, f32)
    bias_idxs = np.asarray(bias_idxs)

    per = 2 * KD + DV  # 192 rows per head in qkv_w
    wq = np.concatenate([qkv_w[h * per : h * per + KD] for h in range(H)], 0)
    wk = np.concatenate([qkv_w[h * per + KD : h * per + 2 * KD] for h in range(H)], 0)
    wv = np.concatenate([qkv_w[h * per + 2 * KD : (h + 1) * per] for h in range(H)], 0)
    bq = np.concatenate([qkv_b[h * per : h * per + KD] for h in range(H)], 0)
    bk = np.concatenate([qkv_b[h * per + KD : h * per + 2 * KD] for h in range(H)], 0)
    bv = np.concatenate([qkv_b[h * per + 2 * KD : (h + 1) * per] for h in range(H)], 0)

    # fold LN affine: xn = xn0 * g + beta  =>  W_eff = W*g, b_eff = W@beta + b
    wq_eff = (wq * ln_g[None, :] * SCALE).astype(f32)
    wk_eff = (wk * ln_g[None, :]).astype(f32)
    wv_eff = (wv * ln_g[None, :]).astype(f32)
    bq_eff = ((wq @ ln_b + bq) * SCALE).astype(f32)
    bk_eff = (wk @ ln_b + bk).astype(f32)
    bv_eff = (wv @ ln_b + bv).astype(f32)

    # group-contiguous packing: [Q 0:512 | Q 512:576 | K 512:576 | K 0:512]
    wqkt = np.concatenate(
        [wq_eff.T[:, 0:512], wq_eff.T[:, 512:576],
         wk_eff.T[:, 512:576], wk_eff.T[:, 0:512]], axis=1
    ).astype(f16).copy()
    wvt = wv_eff.T.astype(f16).copy()
    # proj_w.T packed per head: [128 (dv), 18*576]
    pwp = np.ascontiguousarray(
        proj_w.T.reshape(H, DV, C).transpose(1, 0, 2).reshape(DV, H * C)
    ).astype(f16)

    # packed qk bias: 10 groups [4x128 q, 64 q-tail, 64 k-tail, 4x128 k]
    qkb = np.zeros((128, 10), f32)
    for j in range(4):
        qkb[:, j] = bq_eff[128 * j : 128 * j + 128]
        qkb[:, 6 + j] = bk_eff[128 * j : 128 * j + 128]
    qkb[0:64, 4] = bq_eff[512:576]
    qkb[0:64, 5] = bk_eff[512:576]
    pb = np.zeros((128, NCC), f32)
    for m, (c0, mc) in enumerate(C_CHUNKS):
        pb[:mc, m] = proj_b[c0 : c0 + mc]

    # seed bias, [key, query] per head, overlapping key chunks (0,128) and
    # (68,128); the second chunk's first 60 rows (keys 68:128, already
    # covered by chunk 0) get -60000 so exp gives exact zeros.
    biasT = attn_biases[:, bias_idxs.T]                 # [H, N(key), N(query)]
    biasp = np.zeros((128, H * W), f32)
    for h in range(H):
        biasp[:, h * W : h * W + N] = biasT[h, 0:128, :]
        biasp[0:60, h * W + N : h * W + 2 * N] = -60000.0
        biasp[60:128, h * W + N : h * W + 2 * N] = biasT[h, 128:196, :]
    biasp = biasp.astype(f16)

    has_vbias = bool(np.any(bv_eff != 0.0))
    consts = {
        "wqkt": wqkt, "wvt": wvt, "pwp": pwp,
        "biasp": np.ascontiguousarray(biasp),
        "qkb": qkb, "pb": pb,
    }
    if has_vbias:
        consts["vb"] = np.broadcast_to(bv_eff[None, :], (128, DH)).copy()
    return consts, has_vbias


def kernel(**inputs) -> np.ndarray:
    x = np.asarray(inputs["x"], np.float32)
    consts, has_vbias = _prep_host(
        x, inputs["ln_g"], inputs["ln_b"], inputs["qkv_w"], inputs["qkv_b"],
        inputs["proj_w"], inputs["proj_b"], inputs["attn_biases"],
        inputs["bias_idxs"],
    )
    key = has_vbias
    if key not in _BUILD_CACHE:
        _BUILD_CACHE[key] = _build(has_vbias)
    nc = _BUILD_CACHE[key]

    in_maps = []
    for c in range(NCORES):
        m = {"x": np.ascontiguousarray(x[c * BPC : (c + 1) * BPC])}
        m.update(consts)
        in_maps.append(m)
    res = run_bass_kernel_spmd(nc, in_maps, list(range(NCORES)))
    out = np.concatenate([res.results[c]["out"] for c in range(NCORES)], axis=0)
    return out.astype(np.float32)


# revision 37
# speedup vs baseline: 1.0749x; 1.0749x over previous
"""Trainium2 Bass kernel for the LeViT-style attention block.

Contract: kernel(**inputs) takes the FULL unsharded inputs (numpy) and
returns the FULL [128, 196, 576] float32 output. Internally shards the
batch dim across 8 NeuronCores (16 batches per core) and runs a single
SPMD Bass/Tile program via run_bass_kernel_spmd.

Math (per batch b):
  xn   = LayerNorm(x[b]) * g + beta                     [196, 576]
  qkv  = xn @ qkv_w.T + qkv_b      -> q,k,v per head
  S_h  = (q_h * kd^-0.5) @ k_h.T + bias_h               [196, 196]
  P_h  = softmax(S_h, axis=-1)
  O_h  = P_h @ v_h                                      [196, 128]
  out  = concat_h(O_h) @ proj_w.T + proj_b              [196, 576]

v3 implementation notes (cost-model driven):
  - matmul cost = out-free-size x cycles/row of the MOVING operand
    (fp16/bf16 = 1 always; f32r = 1 only when free >= 256; fp32 = 4);
    contraction depth, partition fill and stationary loads are free, so
    everything 2-byte-able is fp16 (precision) or bf16 (range).
  - token chunks are the OVERLAPPING pairs (0,128) and (68,128): every
    PE output keeps a full, base-0 partition range.  The 60-key overlap
    in the second chunk is killed with -60000 rows in the bias seed
    (exp -> exact 0), so denominator/PV stay correct.
  - scores: per-head bias seeded into PSUM by an identity matmul (fp16,
    196/row), score matmul accumulates on top; exp on ACT; denominator
    via ones-column matmul on PE (PSUM-accumulated across key chunks);
    reciprocal on DVE; ones-row broadcast matmul on PE; normalize fused
    into the PSUM->SBUF O.T copy on DVE.
  - per-head O.T resident in SBUF fp16 (no DRAM scratch); proj streams
    it with fp16 moving operands into 5 resident PSUM banks.
  - weights/bias constants ship as fp16 in a handful of large DMAs on
    the SP (HWDGE) queue, after the first x loads, ordered by first use
    (the Pool/SWDGE queue's per-DMA descriptor-gen cost would starve
    the first superbatches).
  - PSUM->SBUF copies are spread over Pool/ACT/DVE to keep every
    engine under the PE budget.
"""

import os

os.environ.setdefault("MYCRO_LOCAL_CACHE", "1")

from contextlib import ExitStack

import numpy as np
import ml_dtypes

import concourse.bass as bass
import concourse.mybir as mybir
import concourse.tile as tile
from concourse import masks
from concourse.bass import AP
from concourse.bass_utils import run_bass_kernel_spmd

# Problem shape (hardcoded per contest contract).
B, N, C = 128, 196, 576
H, KD, DV = 18, 32, 128
DH = H * DV            # 2304
LN_EPS = 1e-5
SCALE = KD ** -0.5
NCORES = 8
BPC = B // NCORES      # 16 batches per core
SB = 2                 # batches per "superbatch" iteration
NSB = BPC // SB        # 8
W = SB * N             # 392: packed two-batch free dim

FP32 = mybir.dt.float32
F32R = mybir.dt.float32r
FP16 = mybir.dt.float16
BF16 = mybir.dt.bfloat16

# token-dim chunks: overlapping full-128 chunks
TOK_CHUNKS = [(0, 128), (68, 128)]
# C-dim chunks (576 = 4*128 + 64)
C_CHUNKS = [(i * 128, min(128, C - i * 128)) for i in range((C + 127) // 128)]
NCC = len(C_CHUNKS)
# V free-dim chunks of 512 = 4 heads
V_CHUNKS = [(i * 512, min(512, DH - i * 512)) for i in range((DH + 511) // 512)]


def _split_multiwaits(nc):
    """This container's walrus rejects >1 sync-wait per instruction
    (TPB EVENTS struct has a single wait slot). Split extras into
    preceding same-engine NOPs — semantically identical."""
    for f in nc.m.functions:
        for blk in f.blocks:
            newlist = []
            changed = False
            for inst in blk.instructions:
                si = inst.sync_info
                waits = list(si.on_wait) if si is not None else []
                if len(waits) > 1:
                    changed = True
                    for j, w in enumerate(waits[:-1]):
                        nop = mybir.InstNoOp(name=f"{inst.name}_sw{j}", ins=[], outs=[])
                        nop.engine = inst.engine
                        nop.sync_info = mybir.SyncInfo(on_wait=[w], on_update=[])
                        newlist.append(nop)
                    inst.sync_info = mybir.SyncInfo(
                        on_wait=[waits[-1]], on_update=list(si.on_update)
                    )
                newlist.append(inst)
            if changed:
                blk.instructions = newlist


# 10 QK output groups over the packed wqkt columns
# [Q 0:512 | Q 512:576 | K 512:576 | K 0:512]: 4x128 Q, 64 Q-tail,
# 64 K-tail, 4x128 K. Within every group a head's 32 rows sit at the
# same offset for q and k (the PE pairs moving/stationary rows by
# absolute partition, so offsets must match).
QK_GROUPS = [(128 * j, 128) for j in range(4)] + [(512, 64), (576, 64)] + [
    (640 + 128 * j, 128) for j in range(4)
]


def _qk_slice(h):
    """(qgroup, kgroup, offset) for head h."""
    if h < 16:
        return h // 4, 6 + h // 4, (32 * h) % 128
    return 4, 5, 32 * (h - 16)


def _emit(ctx: ExitStack, tc: tile.TileContext, aps: dict, has_vbias: bool):
    nc = tc.nc
    x_d = aps["x"]          # [BPC, 196, 576] f32
    out_d = aps["out"]      # [BPC, 196, 576] f32
    wqk_d = aps["wqkt"]     # [576, 1152] fp16, packed as
                            # [Wq.T rows0:512 | Wq.T 512:576 | Wk.T 512:576 |
                            #  Wk.T rows0:512] so all 9 groups are contiguous
    wv_d = aps["wvt"]       # [576, 2304] fp16 (Wv.T, head-major columns)
    pw_d = aps["pwp"]       # [128, 18*576] fp16 (proj_w.T packed per head)
    bias_d = aps["biasp"]   # [128, 18*392] fp16 (seed bias, chunked
                            #  [head, key-chunk, query]; overlap rows -60000)
    qkb_d = aps["qkb"]      # [128, 9] f32 (per-group qk bias columns)
    pb_d = aps["pb"]        # [128, 5] f32 (proj bias chunks)
    vb_d = aps.get("vb")    # [128, 2304] f32 (replicated v bias) — optional

    cpool = ctx.enter_context(tc.tile_pool(name="consts", bufs=1))
    xpool = ctx.enter_context(tc.tile_pool(name="x", bufs=2))
    xnpool = ctx.enter_context(tc.tile_pool(name="xn", bufs=2))
    stat = ctx.enter_context(tc.tile_pool(name="stat", bufs=2))
    sqpool = ctx.enter_context(tc.tile_pool(name="sq", bufs=2))
    xnt_pool = ctx.enter_context(tc.tile_pool(name="xnt", bufs=1))
    qkt_pool = ctx.enter_context(tc.tile_pool(name="qkt", bufs=1))
    vpool = ctx.enter_context(tc.tile_pool(name="v", bufs=2))
    epool = ctx.enter_context(tc.tile_pool(name="e", bufs=4))
    rbpool = ctx.enter_context(tc.tile_pool(name="rb", bufs=3))
    onpool = ctx.enter_context(tc.tile_pool(name="on", bufs=1))
    ftpool = ctx.enter_context(tc.tile_pool(name="ft", bufs=1))
    fpool = ctx.enter_context(tc.tile_pool(name="f", bufs=2))
    ps = ctx.enter_context(tc.tile_pool(name="ps", bufs=8, space="PSUM"))

    # ---- small on-chip constants ----
    ident = cpool.tile([128, 128], FP16, tag="ident", name="ident")
    masks.make_identity(nc, ident[:])
    allones = cpool.tile([128, 128], BF16, tag="allones", name="allones")
    nc.gpsimd.memset(allones[:], 1.0)
    epsb = cpool.tile([128, 1], FP32, tag="epsb", name="epsb")
    nc.gpsimd.memset(epsb[:], LN_EPS)

    # ---- resident weight tiles (DMAs issued after the first x loads so
    # the SP queue serves x first; ordered by first use) ----
    wqk = [cpool.tile([128, 2 * C], FP16, tag=f"wqk{ci}", name=f"wqk{ci}")
           for ci in range(NCC)]
    wv = [cpool.tile([128, DH], FP16, tag=f"wv{ci}", name=f"wv{ci}")
          for ci in range(NCC)]
    pw = cpool.tile([128, H * C], FP16, tag="pw", name="pw")
    biasT = cpool.tile([128, H * W], FP16, tag="biasT", name="biasT")
    qkb = cpool.tile([128, 10], FP32, tag="qkb", name="qkb")
    pb = cpool.tile([128, NCC], FP32, tag="pb", name="pb")
    vb = None
    if has_vbias:
        vb = cpool.tile([128, DH], FP32, tag="vb", name="vb")

    def load_weights():
        for ci, (c0, cs) in enumerate(C_CHUNKS):
            nc.sync.dma_start(wqk[ci][:cs, :], wqk_d[c0 : c0 + cs, :])
        nc.sync.dma_start(qkb[:], qkb_d[:])
        nc.sync.dma_start(biasT[:], bias_d[:])
        for ci, (c0, cs) in enumerate(C_CHUNKS):
            nc.sync.dma_start(wv[ci][:cs, :], wv_d[c0 : c0 + cs, :])
        if has_vbias:
            nc.sync.dma_start(vb[:], vb_d[:])
        nc.sync.dma_start(pw[:], pw_d[:])
        nc.sync.dma_start(pb[:], pb_d[:])

    inv_c = 1.0 / C

    lnstate = {}
    lnmid = {}
    xstate = {}

    def ln_load(sbx, q, tj):
        b = sbx * SB + q
        t0, ts_ = TOK_CHUNKS[tj]
        xt = xpool.tile([128, C], FP32, tag=f"xb{q}{tj}", name=f"xb{q}{tj}_{sbx}")
        nc.sync.dma_start(xt[:ts_, :], x_d[b, t0 : t0 + ts_, :])
        xstate[(sbx, q, tj)] = xt

    def ln_chunk(sbx, q, tj):
        ln_stats(sbx, q, tj)
        ln_apply(sbx, q, tj)

    def ln_stats(sbx, q, tj):
        """LayerNorm stats for one loaded (batch, token-chunk)."""
        t0, ts_ = TOK_CHUNKS[tj]
        xt = xstate.pop((sbx, q, tj))
        negmu = stat.tile([128, 1], FP32, tag="negmu", name=f"nm{q}{tj}_{sbx}")
        nc.vector.tensor_reduce(
            negmu[:ts_], xt[:ts_, :], axis=mybir.AxisListType.X,
            op=mybir.AluOpType.add, negate=True,
        )
        nc.vector.tensor_scalar_mul(negmu[:ts_], negmu[:ts_], inv_c)
        sq = sqpool.tile([128, C], FP32, tag="sq", name=f"sq{q}{tj}_{sbx}")
        ha = stat.tile([128, 1], FP32, tag="ha", name=f"ha{q}{tj}_{sbx}")
        nc.scalar.activation(
            sq[:ts_, :], xt[:ts_, :],
            mybir.ActivationFunctionType.Square, bias=negmu[:ts_], accum_out=ha[:ts_],
        )
        std = stat.tile([128, 1], FP32, tag="std", name=f"std{q}{tj}_{sbx}")
        nc.scalar.activation(
            std[:ts_], ha[:ts_], mybir.ActivationFunctionType.Sqrt,
            bias=epsb[:ts_], scale=inv_c,
        )
        r = stat.tile([128, 1], FP32, tag="r", name=f"r{q}{tj}_{sbx}")
        nc.vector.reciprocal(r[:ts_], std[:ts_])
        negmur = stat.tile([128, 1], FP32, tag="negmur", name=f"nr{q}{tj}_{sbx}")
        nc.vector.tensor_mul(negmur[:ts_], negmu[:ts_], r[:ts_])
        lnmid[(sbx, q, tj)] = (xt, negmur, r)

    def ln_apply(sbx, q, tj):
        """normalize in fp16 from the precomputed stats."""
        t0, ts_ = TOK_CHUNKS[tj]
        xt, negmur, r = lnmid.pop((sbx, q, tj))
        xn = xnpool.tile([128, C], FP16, tag=f"xn{q}{tj}", name=f"xn{q}{tj}_{sbx}")
        nc.vector.tensor_scalar(
            xn[:ts_, :], xt[:ts_, :], r[:ts_], negmur[:ts_],
            op0=mybir.AluOpType.mult, op1=mybir.AluOpType.add,
        )
        lnstate[(sbx, q, tj)] = xn

    pending_finals = []

    for q in range(SB):
        for tj in range(len(TOK_CHUNKS)):
            ln_load(0, q, tj)
    load_weights()
    for q in range(SB):
        for tj in range(len(TOK_CHUNKS)):
            ln_chunk(0, q, tj)

    xnt_staged = {}

    def emit_transpose(sbx, q, tj, xnt):
        t0, ts_ = TOK_CHUNKS[tj]
        xn = lnstate.pop((sbx, q, tj))
        for ci, (c0, cs) in enumerate(C_CHUNKS):
            pt = ps.tile([128, 512], FP16, tag="ps", name=f"pst{q}{tj}{ci}_{sbx}")
            nc.tensor.transpose(
                pt[:cs, :ts_], xn[:ts_, c0 : c0 + cs], ident[:ts_, :ts_]
            )
            col = q * N + t0
            if ci % 2 == 0:
                nc.scalar.copy(xnt[ci][:cs, col : col + ts_], pt[:cs, :ts_])
            else:
                nc.vector.tensor_copy(xnt[ci][:cs, col : col + ts_], pt[:cs, :ts_])

    for sb in range(NSB):
        # ---- xn -> xnT (channel-major): staged during the previous
        # superbatch's proj pass, except for the first superbatch ----
        if sb in xnt_staged:
            xnt = xnt_staged.pop(sb)
        else:
            xnt = [
                xnt_pool.tile([128, W], FP16, tag=f"xnt{ci}", name=f"xnt{ci}_{sb}")
                for ci in range(NCC)
            ]
            for q in range(SB):
                for tj in range(len(TOK_CHUNKS)):
                    emit_transpose(sb, q, tj, xnt)

        if sb + 1 < NSB:
            for q in range(SB):
                for tj in range(len(TOK_CHUNKS)):
                    ln_load(sb + 1, q, tj)

        # ---- Q.T / K.T GEMMs (9 groups of 128 rows, both batches) ----
        qkt = [None] * 10
        for n_, j in enumerate((0, 6, 1, 7, 2, 8, 3, 9, 4, 5)):
            col0, rows = QK_GROUPS[j]
            pq = ps.tile([128, 512], FP32, tag="ps", name=f"psqk{j}_{sb}")
            for ci, (c0, cs) in enumerate(C_CHUNKS):
                nc.tensor.matmul(
                    pq[:rows, :W], wqk[ci][:cs, col0 : col0 + rows],
                    xnt[ci][:cs, :W],
                    start=(ci == 0), stop=(ci == NCC - 1),
                )
            t = qkt_pool.tile([128, W], FP16, tag=f"qkt{j}", name=f"qkt{j}_{sb}")
            if n_ % 2 == 0:
                nc.vector.tensor_scalar_add(
                    t[:rows, :], pq[:rows, :W], qkb[:rows, j : j + 1]
                )
            else:
                nc.scalar.activation(
                    t[:rows, :], pq[:rows, :W],
                    mybir.ActivationFunctionType.Identity, bias=qkb[:rows, j : j + 1],
                )
            qkt[j] = t

        # ---- attention: software-pipelined head loop ----
        vgroups = {}
        estate = {}
        b1state = {}
        onorm = [
            onpool.tile([128, W], FP16, tag=f"on{h}", name=f"on{h}_{sb}")
            for h in range(H)
        ]

        def emit_v_chunk(g, k, sb=sb, xnt=xnt, vgroups=vgroups):
            n0, ns = V_CHUNKS[g]
            q, tj = k // 2, k % 2
            t0, ts_ = TOK_CHUNKS[tj]
            v = vpool.tile([128, 512], BF16, tag=f"v{q}{tj}",
                           name=f"v{q}{tj}g{g}_{sb}")
            pv = ps.tile([128, 512], FP32, tag="ps", name=f"psv{q}{tj}{g}_{sb}")
            for ci, (c0, cs) in enumerate(C_CHUNKS):
                nc.tensor.matmul(
                    pv[:ts_, :ns], xnt[ci][:cs, q * N + t0 : q * N + t0 + ts_],
                    wv[ci][:cs, n0 : n0 + ns],
                    start=(ci == 0), stop=(ci == NCC - 1),
                )
            if has_vbias:
                nc.vector.tensor_add(
                    v[:ts_, :ns], pv[:ts_, :ns], vb[:ts_, n0 : n0 + ns]
                )
            elif (q + tj) % 2 == 0:
                nc.scalar.copy(v[:ts_, :ns], pv[:ts_, :ns])
            else:
                nc.vector.tensor_copy(v[:ts_, :ns], pv[:ts_, :ns])
            vgroups.setdefault(g, {})[(q, tj)] = v

        def stage_a(h, sb=sb, qkt=qkt, estate=estate):
            # seeds + scores + exp for head h; S.T in [key, query] layout
            qg, kg, ro = _qk_slice(h)
            qt, kt = qkt[qg], qkt[kg]
            st = []
            for q in range(SB):
                s = ps.tile([128, 512], FP32, tag="ps", name=f"st{q}h{h}_{sb}")
                for tj, (t0, ts_) in enumerate(TOK_CHUNKS):
                    nc.tensor.matmul(
                        s[:128, tj * N : tj * N + N],
                        ident[:128, :],
                        biasT[:, (h * 2 + tj) * N : (h * 2 + tj) * N + N],
                        start=True, stop=False, skip_group_check=True,
                    )
                    nc.tensor.matmul(
                        s[:128, tj * N : tj * N + N],
                        kt[ro : ro + 32, q * N + t0 : q * N + t0 + ts_],
                        qt[ro : ro + 32, q * N : q * N + N],
                        start=False, stop=True, skip_group_check=True,
                        tile_position=(ro, 0),
                    )
                st.append(s)
            # e layout: [key-chunk, batch, query] so each key-chunk's
            # 392 columns are contiguous (dn needs a 1-D moving AP)
            e = epool.tile([128, 2 * W], BF16, tag="e", name=f"e{h}_{sb}")
            e4 = e[:, :].rearrange("p (c q n) -> p c q n", c=2, q=2)
            for q in range(SB):
                st2 = st[q][:, 0 : 2 * N].rearrange("p (c n) -> p c n", c=2)
                nc.scalar.activation(e4[:, :, q], st2,
                                     mybir.ActivationFunctionType.Exp)
            estate[h] = e

        def stage_b1(h, sb=sb, estate=estate, b1state=b1state):
            # denominator, already broadcast: an all-ones stationary gives
            # every output partition the key-sum in one accumulated matmul
            e = estate[h]
            bcp = ps.tile([128, W], FP32, tag="ps", name=f"bcp{h}_{sb}")
            for tj in range(2):
                nc.tensor.matmul(
                    bcp[:, :W], allones[:, :], e[:, tj * W : tj * W + W],
                    start=(tj == 0), stop=(tj == 1),
                )
            rb = rbpool.tile([128, W], FP32, tag="rb", name=f"rb{h}_{sb}")
            nc.vector.reciprocal(rb[:, :], bcp[:, :W])
            b1state[h] = rb

        def stage_pv(h, sb=sb, estate=estate, vgroups=vgroups):
            e = estate.pop(h)
            g = h // 4
            vt = vgroups[g]
            n0, ns = V_CHUNKS[g]
            hcol = h * DV - n0
            ou = ps.tile([128, 512], FP32, tag="ps", name=f"ou{h}_{sb}")
            for q in range(SB):
                for tj, (t0, ts_) in enumerate(TOK_CHUNKS):
                    nc.tensor.matmul(
                        ou[:DV, q * N : q * N + N],
                        vt[(q, tj)][:ts_, hcol : hcol + DV],
                        e[:, tj * W + q * N : tj * W + q * N + N],
                        start=(tj == 0), stop=(tj == 1),
                    )
            return ou

        def stage_b3(h, ou, sb=sb, b1state=b1state, onorm=onorm):
            # normalize O.T into its resident fp16 slot (DVE)
            rb = b1state.pop(h)
            nc.vector.tensor_mul(
                onorm[h][:, :].rearrange("p (b n) -> p b n", b=2, n=N),
                ou[:DV, 0 : 2 * N].rearrange("p (b n) -> p b n", b=2, n=N),
                rb[:, :].rearrange("p (b n) -> p b n", b=2, n=N),
            )

        # pipeline: dn/recip lead by 1 head, broadcast trails so the PE
        # queue never waits on the DVE reciprocal
        for k in range(4):
            emit_v_chunk(0, k)
        stage_a(0)
        stage_a(1)
        stage_b1(0)
        for h in range(H):
            g_next = h // 4 + 1
            if g_next <= 4:
                emit_v_chunk(g_next, h % 4)
            if h + 2 < H:
                stage_a(h + 2)
            if h + 1 < H:
                stage_b1(h + 1)
            ou = stage_pv(h)
            stage_b3(h, ou)
            if pending_finals and h in (0, 1, 2, 4):
                pending_finals.pop(0)()
            if sb + 1 < NSB:
                if h in (5, 7, 9, 11):
                    k = (5, 7, 9, 11).index(h)
                    ln_stats(sb + 1, k // 2, k % 2)
                elif h in (6, 8, 10, 12):
                    k = (6, 8, 10, 12).index(h)
                    ln_apply(sb + 1, k // 2, k % 2)

        # ---- proj: accumulate heads from resident O.T, with the next
        # superbatch's xn transposes interleaved (PE stays busy on proj
        # while the transpose copies drain on ACT/DVE) ----
        ft = ftpool.tile([128, NCC * W], FP16, tag="ft", name=f"ft{sb}")
        if sb + 1 < NSB:
            xnt_next = [
                xnt_pool.tile([128, W], FP16, tag=f"xnt{ci}", name=f"xnt{ci}_{sb + 1}")
                for ci in range(NCC)
            ]
            xnt_staged[sb + 1] = xnt_next
        # m-outer / h-inner: one PSUM bank at a time (onorm re-reads from
        # SBUF are free), leaving banks for the interleaved transposes
        for m in range(NCC):
            c0, mc = C_CHUNKS[m]
            if sb + 1 < NSB and m < 4:
                emit_transpose(sb + 1, m // 2, m % 2, xnt_next)
            pp = ps.tile([128, W], FP32, tag="ps", name=f"pp{m}_{sb}")
            for h in range(H):
                nc.tensor.matmul(
                    pp[:mc, :W], pw[:, h * C + c0 : h * C + c0 + mc],
                    onorm[h][:DV, :W],
                    start=(h == 0), stop=(h == H - 1),
                )
            nc.vector.tensor_scalar_add(
                ft[:mc, m * W : m * W + W], pp[:mc, :W], pb[:mc, m : m + 1]
            )

        # ---- transpose back to token layout and store (deferred into the
        # next superbatch's head loop; flushed immediately on the last) ----
        def make_final(q, tj, ft=ft, sb=sb):
            def emit():
                b = sb * SB + q
                t0, ts_ = TOK_CHUNKS[tj]
                f = fpool.tile([128, C], FP32, tag="f", name=f"f{q}{tj}_{sb}")
                for m, (c0, mc) in enumerate(C_CHUNKS):
                    pt = ps.tile([128, 512], FP16, tag="ps", name=f"psf{m}{q}{tj}_{sb}")
                    src0 = m * W + q * N + t0
                    nc.tensor.transpose(
                        pt[:ts_, :mc], ft[:mc, src0 : src0 + ts_], ident[:mc, :mc]
                    )
                    if m % 2 == 0:
                        nc.scalar.copy(f[:ts_, c0 : c0 + mc], pt[:ts_, :mc])
                    else:
                        nc.vector.tensor_copy(f[:ts_, c0 : c0 + mc], pt[:ts_, :mc])
                nc.sync.dma_start(out_d[b, t0 : t0 + ts_, :], f[:ts_, :])
            return emit

        chunks = [make_final(q, tj) for q in range(SB) for tj in range(2)]
        if sb + 1 < NSB:
            pending_finals.extend(chunks)
        else:
            for c in chunks:
                c()


def _build(has_vbias: bool):
    nc = bass.Bass(
        "TRN2", target_bir_lowering=False, debug=False,
        enable_asserts=False, num_devices=NCORES,
    )
    aps = {}
    aps["x"] = nc.dram_tensor("x", [BPC, N, C], FP32, kind="ExternalInput").ap()
    aps["wqkt"] = nc.dram_tensor("wqkt", [C, 2 * C], FP16, kind="ExternalInput").ap()
    aps["wvt"] = nc.dram_tensor("wvt", [C, DH], FP16, kind="ExternalInput").ap()
    aps["pwp"] = nc.dram_tensor("pwp", [128, H * C], FP16, kind="ExternalInput").ap()
    aps["biasp"] = nc.dram_tensor("biasp", [128, H * W], FP16, kind="ExternalInput").ap()
    aps["qkb"] = nc.dram_tensor("qkb", [128, 10], FP32, kind="ExternalInput").ap()
    aps["pb"] = nc.dram_tensor("pb", [128, NCC], FP32, kind="ExternalInput").ap()
    if has_vbias:
        aps["vb"] = nc.dram_tensor("vb", [128, DH], FP32, kind="ExternalInput").ap()
    aps["out"] = nc.dram_tensor("out", [BPC, N, C], FP32, kind="ExternalOutput").ap()

    with tile.TileContext(nc) as tc, ExitStack() as ctx:
        with nc.allow_low_precision(reason="fp16/bf16 matmul pipeline"):
            _emit(ctx, tc, aps, has_vbias)
    _split_multiwaits(nc)
    return nc


_BUILD_CACHE: dict = {}


def _prep_host(x, ln_g, ln_b, qkv_w, qkv_b, proj_w, proj_b, attn_biases, bias_idxs):
    """Permute/fold weights host-side. Returns (in_map_consts, has_vbias)."""
    f32 = np.float32
    f16 = np.float16
    qkv_w = np.asarray(qkv_w, f32)
    qkv_b = np.asarray(qkv_b, f32)
    ln_g = np.asarray(ln_g, f32)
    ln_b = np.asarray(ln_b, f32)
    proj_w = np.asarray(proj_w, f32)
    proj_b = np.asarray(proj_b, f32)
    attn_biases = np.asarray(attn_biases, f32)
    bias_idxs = np.asarray(bias_idxs)

    per = 2 * KD + DV  # 192 rows per head in qkv_w
    wq = np.concatenate([qkv_w[h * per : h * per + KD] for h in range(H)], 0)
    wk = np.concatenate([qkv_w[h * per + KD : h * per + 2 * KD] for h in range(H)], 0)
    wv = np.concatenate([qkv_w[h * per + 2 * KD : (h + 1) * per] for h in range(H)], 0)
    bq = np.concatenate([qkv_b[h * per : h * per + KD] for h in range(H)], 0)
    bk = np.concatenate([qkv_b[h * per + KD : h * per + 2 * KD] for h in range(H)], 0)
    bv = np.concatenate([qkv_b[h * per + 2 * KD : (h + 1) * per] for h in range(H)], 0)

    # fold LN affine: xn = xn0 * g + beta  =>  W_eff = W*g, b_eff = W@beta + b
    wq_eff = (wq * ln_g[None, :] * SCALE).astype(f32)
    wk_eff = (wk * ln_g[None, :]).astype(f32)
    wv_eff = (wv * ln_g[None, :]).astype(f32)
    bq_eff = ((wq @ ln_b + bq) * SCALE).astype(f32)
    bk_eff = (wk @ ln_b + bk).astype(f32)
    bv_eff = (wv @ ln_b + bv).astype(f32)

    # group-contiguous packing: [Q 0:512 | Q 512:576 | K 512:576 | K 0:512]
    wqkt = np.concatenate(
        [wq_eff.T[:, 0:512], wq_eff.T[:, 512:576],
         wk_eff.T[:, 512:576], wk_eff.T[:, 0:512]], axis=1
    ).astype(f16).copy()
    wvt = wv_eff.T.astype(f16).copy()
    # proj_w.T packed per head: [128 (dv), 18*576]
    pwp = np.ascontiguousarray(
        proj_w.T.reshape(H, DV, C).transpose(1, 0, 2).reshape(DV, H * C)
    ).astype(f16)

    # packed qk bias: 10 groups [4x128 q, 64 q-tail, 64 k-tail, 4x128 k]
    qkb = np.zeros((128, 10), f32)
    for j in range(4):
        qkb[:, j] = bq_eff[128 * j : 128 * j + 128]
        qkb[:, 6 + j] = bk_eff[128 * j : 128 * j + 128]
    qkb[0:64, 4] = bq_eff[512:576]
    qkb[0:64, 5] = bk_eff[512:576]
    pb = np.zeros((128, NCC), f32)
    for m, (c0, mc) in enumerate(C_CHUNKS):
        pb[:mc, m] = proj_b[c0 : c0 + mc]

    # seed bias, [key, query] per head, overlapping key chunks (0,128) and
    # (68,128); the second chunk's first 60 rows (keys 68:128, already
    # covered by chunk 0) get -60000 so exp gives exact zeros.
    biasT = attn_biases[:, bias_idxs.T]                 # [H, N(key), N(query)]
    biasp = np.zeros((128, H * W), f32)
    for h in range(H):
        biasp[:, h * W : h * W + N] = biasT[h, 0:128, :]
        biasp[0:60, h * W + N : h * W + 2 * N] = -60000.0
        biasp[60:128, h * W + N : h * W + 2 * N] = biasT[h, 128:196, :]
    biasp = biasp.astype(f16)

    has_vbias = bool(np.any(bv_eff != 0.0))
    consts = {
        "wqkt": wqkt, "wvt": wvt, "pwp": pwp,
        "biasp": np.ascontiguousarray(biasp),
        "qkb": qkb, "pb": pb,
    }
    if has_vbias:
        consts["vb"] = np.broadcast_to(bv_eff[None, :], (128, DH)).copy()
    return consts, has_vbias


def kernel(**inputs) -> np.ndarray:
    x = np.asarray(inputs["x"], np.float32)
    consts, has_vbias = _prep_host(
        x, inputs["ln_g"], inputs["ln_b"], inputs["qkv_w"], inputs["qkv_b"],
        inputs["proj_w"], inputs["proj_b"], inputs["attn_biases"],
        inputs["bias_idxs"],
    )
    key = has_vbias
    if key not in _BUILD_CACHE:
        _BUILD_CACHE[key] = _build(has_vbias)
    nc = _BUILD_CACHE[key]

    in_maps = []
    for c in range(NCORES):
        m = {"x": np.ascontiguousarray(x[c * BPC : (c + 1) * BPC])}
        m.update(consts)
        in_maps.append(m)
    res = run_bass_kernel_spmd(nc, in_maps, list(range(NCORES)))
    out = np.concatenate([res.results[c]["out"] for c in range(NCORES)], axis=0)
    return out.astype(np.float32)


# revision 39
# speedup vs baseline: 1.0877x; 1.0119x over previous
"""Trainium2 Bass kernel for the LeViT-style attention block.

Contract: kernel(**inputs) takes the FULL unsharded inputs (numpy) and
returns the FULL [128, 196, 576] float32 output. Internally shards the
batch dim across 8 NeuronCores (16 batches per core) and runs a single
SPMD Bass/Tile program via run_bass_kernel_spmd.

Math (per batch b):
  xn   = LayerNorm(x[b]) * g + beta                     [196, 576]
  qkv  = xn @ qkv_w.T + qkv_b      -> q,k,v per head
  S_h  = (q_h * kd^-0.5) @ k_h.T + bias_h               [196, 196]
  P_h  = softmax(S_h, axis=-1)
  O_h  = P_h @ v_h                                      [196, 128]
  out  = concat_h(O_h) @ proj_w.T + proj_b              [196, 576]

v3 implementation notes (cost-model driven):
  - matmul cost = out-free-size x cycles/row of the MOVING operand
    (fp16/bf16 = 1 always; f32r = 1 only when free >= 256; fp32 = 4);
    contraction depth, partition fill and stationary loads are free, so
    everything 2-byte-able is fp16 (precision) or bf16 (range).
  - token chunks are the OVERLAPPING pairs (0,128) and (68,128): every
    PE output keeps a full, base-0 partition range.  The 60-key overlap
    in the second chunk is killed with -60000 rows in the bias seed
    (exp -> exact 0), so denominator/PV stay correct.
  - scores: per-head bias seeded into PSUM by an identity matmul (fp16,
    196/row), score matmul accumulates on top; exp on ACT; denominator
    via ones-column matmul on PE (PSUM-accumulated across key chunks);
    reciprocal on DVE; ones-row broadcast matmul on PE; normalize fused
    into the PSUM->SBUF O.T copy on DVE.
  - per-head O.T resident in SBUF fp16 (no DRAM scratch); proj streams
    it with fp16 moving operands into 5 resident PSUM banks.
  - weights/bias constants ship as fp16 in a handful of large DMAs on
    the SP (HWDGE) queue, after the first x loads, ordered by first use
    (the Pool/SWDGE queue's per-DMA descriptor-gen cost would starve
    the first superbatches).
  - PSUM->SBUF copies are spread over Pool/ACT/DVE to keep every
    engine under the PE budget.
"""

import os

os.environ.setdefault("MYCRO_LOCAL_CACHE", "1")

from contextlib import ExitStack

import numpy as np
import ml_dtypes

import concourse.bass as bass
import concourse.mybir as mybir
import concourse.tile as tile
from concourse import masks
from concourse.bass import AP
from concourse.bass_utils import run_bass_kernel_spmd

# Problem shape (hardcoded per contest contract).
B, N, C = 128, 196, 576
H, KD, DV = 18, 32, 128
DH = H * DV            # 2304
LN_EPS = 1e-5
SCALE = KD ** -0.5
NCORES = 8
BPC = B // NCORES      # 16 batches per core
SB = 2                 # batches per "superbatch" iteration
NSB = BPC // SB        # 8
W = SB * N             # 392: packed two-batch free dim

FP32 = mybir.dt.float32
F32R = mybir.dt.float32r
FP16 = mybir.dt.float16
BF16 = mybir.dt.bfloat16

# token-dim chunks: overlapping full-128 chunks
TOK_CHUNKS = [(0, 128), (68, 128)]
# C-dim chunks (576 = 4*128 + 64)
C_CHUNKS = [(i * 128, min(128, C - i * 128)) for i in range((C + 127) // 128)]
NCC = len(C_CHUNKS)
# V free-dim chunks of 512 = 4 heads
V_CHUNKS = [(i * 512, min(512, DH - i * 512)) for i in range((DH + 511) // 512)]


def _split_multiwaits(nc):
    """This container's walrus rejects >1 sync-wait per instruction
    (TPB EVENTS struct has a single wait slot). Split extras into
    preceding same-engine NOPs — semantically identical."""
    for f in nc.m.functions:
        for blk in f.blocks:
            newlist = []
            changed = False
            for inst in blk.instructions:
                si = inst.sync_info
                waits = list(si.on_wait) if si is not None else []
                if len(waits) > 1:
                    changed = True
                    for j, w in enumerate(waits[:-1]):
                        nop = mybir.InstNoOp(name=f"{inst.name}_sw{j}", ins=[], outs=[])
                        nop.engine = inst.engine
                        nop.sync_info = mybir.SyncInfo(on_wait=[w], on_update=[])
                        newlist.append(nop)
                    inst.sync_info = mybir.SyncInfo(
                        on_wait=[waits[-1]], on_update=list(si.on_update)
                    )
                newlist.append(inst)
            if changed:
                blk.instructions = newlist


# 10 QK output groups over the packed wqkt columns
# [Q 0:512 | Q 512:576 | K 512:576 | K 0:512]: 4x128 Q, 64 Q-tail,
# 64 K-tail, 4x128 K. Within every group a head's 32 rows sit at the
# same offset for q and k (the PE pairs moving/stationary rows by
# absolute partition, so offsets must match).
QK_GROUPS = [(128 * j, 128) for j in range(4)] + [(512, 128)] + [
    (640 + 128 * j, 128) for j in range(4)
]


def _qk_slice(h):
    """(qgroup, kgroup, offset) for head h. Group 4 holds [Q-tail;
    K-tail]; the K-tail is DMA-shifted to partition 0 in tile 9 so the
    per-head q/k offsets match."""
    if h < 16:
        return h // 4, 5 + h // 4, (32 * h) % 128
    return 4, 9, 32 * (h - 16)


def _emit(ctx: ExitStack, tc: tile.TileContext, aps: dict, has_vbias: bool):
    nc = tc.nc
    x_d = aps["x"]          # [BPC, 196, 576] f32
    out_d = aps["out"]      # [BPC, 196, 576] f32
    wqk_d = aps["wqkt"]     # [576, 1152] fp16, packed as
                            # [Wq.T rows0:512 | Wq.T 512:576 | Wk.T 512:576 |
                            #  Wk.T rows0:512] so all 9 groups are contiguous
    wv_d = aps["wvt"]       # [576, 2304] fp16 (Wv.T, head-major columns)
    pw_d = aps["pwp"]       # [128, 18*576] fp16 (proj_w.T packed per head)
    bias_d = aps["biasp"]   # [128, 18*392] fp16 (seed bias, chunked
                            #  [head, key-chunk, query]; overlap rows -60000)
    qkb_d = aps["qkb"]      # [128, 9] f32 (per-group qk bias columns)
    pb_d = aps["pb"]        # [128, 5] f32 (proj bias chunks)
    vb_d = aps.get("vb")    # [128, 2304] f32 (replicated v bias) — optional

    cpool = ctx.enter_context(tc.tile_pool(name="consts", bufs=1))
    xpool = ctx.enter_context(tc.tile_pool(name="x", bufs=2))
    xnpool = ctx.enter_context(tc.tile_pool(name="xn", bufs=2))
    stat = ctx.enter_context(tc.tile_pool(name="stat", bufs=2))
    sqpool = ctx.enter_context(tc.tile_pool(name="sq", bufs=2))
    xnt_pool = ctx.enter_context(tc.tile_pool(name="xnt", bufs=1))
    qkt_pool = ctx.enter_context(tc.tile_pool(name="qkt", bufs=1))
    vpool = ctx.enter_context(tc.tile_pool(name="v", bufs=2))
    epool = ctx.enter_context(tc.tile_pool(name="e", bufs=4))
    rbpool = ctx.enter_context(tc.tile_pool(name="rb", bufs=3))
    onpool = ctx.enter_context(tc.tile_pool(name="on", bufs=1))
    ftpool = ctx.enter_context(tc.tile_pool(name="ft", bufs=1))
    fpool = ctx.enter_context(tc.tile_pool(name="f", bufs=2))
    ps = ctx.enter_context(tc.tile_pool(name="ps", bufs=8, space="PSUM"))

    # ---- small on-chip constants ----
    ident = cpool.tile([128, 128], FP16, tag="ident", name="ident")
    masks.make_identity(nc, ident[:])
    allones = cpool.tile([128, 128], BF16, tag="allones", name="allones")
    nc.gpsimd.memset(allones[:], 1.0)
    epsb = cpool.tile([128, 1], FP32, tag="epsb", name="epsb")
    nc.gpsimd.memset(epsb[:], LN_EPS)

    # ---- resident weight tiles (DMAs issued after the first x loads so
    # the SP queue serves x first; ordered by first use) ----
    wqk = [cpool.tile([128, 2 * C], FP16, tag=f"wqk{ci}", name=f"wqk{ci}")
           for ci in range(NCC)]
    wv = [cpool.tile([128, DH], FP16, tag=f"wv{ci}", name=f"wv{ci}")
          for ci in range(NCC)]
    pw = cpool.tile([128, H * C], FP16, tag="pw", name="pw")
    biasT = cpool.tile([128, H * W], FP16, tag="biasT", name="biasT")
    qkb = cpool.tile([128, 10], FP32, tag="qkb", name="qkb")
    pb = cpool.tile([128, NCC], FP32, tag="pb", name="pb")
    vb = None
    if has_vbias:
        vb = cpool.tile([128, DH], FP32, tag="vb", name="vb")

    def load_weights():
        for ci, (c0, cs) in enumerate(C_CHUNKS):
            nc.sync.dma_start(wqk[ci][:cs, :], wqk_d[c0 : c0 + cs, :])
        nc.sync.dma_start(qkb[:], qkb_d[:])
        nc.sync.dma_start(biasT[:], bias_d[:])
        for ci, (c0, cs) in enumerate(C_CHUNKS):
            nc.sync.dma_start(wv[ci][:cs, :], wv_d[c0 : c0 + cs, :])
        if has_vbias:
            nc.sync.dma_start(vb[:], vb_d[:])
        nc.sync.dma_start(pw[:], pw_d[:])
        nc.sync.dma_start(pb[:], pb_d[:])

    inv_c = 1.0 / C

    lnstate = {}
    lnmid = {}
    xstate = {}

    def ln_load(sbx, q, tj):
        b = sbx * SB + q
        t0, ts_ = TOK_CHUNKS[tj]
        xt = xpool.tile([128, C], FP32, tag=f"xb{q}{tj}", name=f"xb{q}{tj}_{sbx}")
        nc.sync.dma_start(xt[:ts_, :], x_d[b, t0 : t0 + ts_, :])
        xstate[(sbx, q, tj)] = xt

    def ln_chunk(sbx, q, tj):
        ln_stats(sbx, q, tj)
        ln_apply(sbx, q, tj)

    def ln_stats(sbx, q, tj):
        """LayerNorm stats for one loaded (batch, token-chunk)."""
        t0, ts_ = TOK_CHUNKS[tj]
        xt = xstate.pop((sbx, q, tj))
        negmu = stat.tile([128, 1], FP32, tag="negmu", name=f"nm{q}{tj}_{sbx}")
        nc.vector.tensor_reduce(
            negmu[:ts_], xt[:ts_, :], axis=mybir.AxisListType.X,
            op=mybir.AluOpType.add, negate=True,
        )
        nc.vector.tensor_scalar_mul(negmu[:ts_], negmu[:ts_], inv_c)
        sq = sqpool.tile([128, C], FP32, tag="sq", name=f"sq{q}{tj}_{sbx}")
        ha = stat.tile([128, 1], FP32, tag="ha", name=f"ha{q}{tj}_{sbx}")
        nc.scalar.activation(
            sq[:ts_, :], xt[:ts_, :],
            mybir.ActivationFunctionType.Square, bias=negmu[:ts_], accum_out=ha[:ts_],
        )
        std = stat.tile([128, 1], FP32, tag="std", name=f"std{q}{tj}_{sbx}")
        nc.scalar.activation(
            std[:ts_], ha[:ts_], mybir.ActivationFunctionType.Sqrt,
            bias=epsb[:ts_], scale=inv_c,
        )
        r = stat.tile([128, 1], FP32, tag="r", name=f"r{q}{tj}_{sbx}")
        nc.vector.reciprocal(r[:ts_], std[:ts_])
        negmur = stat.tile([128, 1], FP32, tag="negmur", name=f"nr{q}{tj}_{sbx}")
        nc.vector.tensor_mul(negmur[:ts_], negmu[:ts_], r[:ts_])
        lnmid[(sbx, q, tj)] = (xt, negmur, r)

    def ln_apply(sbx, q, tj):
        """normalize in fp16 from the precomputed stats."""
        t0, ts_ = TOK_CHUNKS[tj]
        xt, negmur, r = lnmid.pop((sbx, q, tj))
        xn = xnpool.tile([128, C], FP16, tag=f"xn{q}{tj}", name=f"xn{q}{tj}_{sbx}")
        nc.vector.tensor_scalar(
            xn[:ts_, :], xt[:ts_, :], r[:ts_], negmur[:ts_],
            op0=mybir.AluOpType.mult, op1=mybir.AluOpType.add,
        )
        lnstate[(sbx, q, tj)] = xn

    pending_finals = []

    for q in range(SB):
        for tj in range(len(TOK_CHUNKS)):
            ln_load(0, q, tj)
    load_weights()
    for q in range(SB):
        for tj in range(len(TOK_CHUNKS)):
            ln_chunk(0, q, tj)

    xnt_staged = {}
    vstaged = {}
    vstaged_done = set()

    def emit_transpose(sbx, q, tj, xnt):
        t0, ts_ = TOK_CHUNKS[tj]
        xn = lnstate.pop((sbx, q, tj))
        for ci, (c0, cs) in enumerate(C_CHUNKS):
            pt = ps.tile([128, 512], FP16, tag="ps", name=f"pst{q}{tj}{ci}_{sbx}")
            nc.tensor.transpose(
                pt[:cs, :ts_], xn[:ts_, c0 : c0 + cs], ident[:ts_, :ts_]
            )
            col = q * N + t0
            if ci % 2 == 0:
                nc.scalar.copy(xnt[ci][:cs, col : col + ts_], pt[:cs, :ts_])
            else:
                nc.vector.tensor_copy(xnt[ci][:cs, col : col + ts_], pt[:cs, :ts_])

    for sb in range(NSB):
        # ---- xn -> xnT (channel-major): staged during the previous
        # superbatch's proj pass, except for the first superbatch ----
        if sb in xnt_staged:
            xnt = xnt_staged.pop(sb)
        else:
            xnt = [
                xnt_pool.tile([128, W], FP16, tag=f"xnt{ci}", name=f"xnt{ci}_{sb}")
                for ci in range(NCC)
            ]
            for q in range(SB):
                for tj in range(len(TOK_CHUNKS)):
                    emit_transpose(sb, q, tj, xnt)

        if sb + 1 < NSB:
            for q in range(SB):
                for tj in range(len(TOK_CHUNKS)):
                    ln_load(sb + 1, q, tj)

        # ---- Q.T / K.T GEMMs (9 groups of 128 rows, both batches) ----
        qkt = [None] * 10
        for n_, j in enumerate((0, 5, 1, 6, 2, 7, 3, 8, 4)):
            col0, rows = QK_GROUPS[j]
            pq = ps.tile([128, 512], FP32, tag="ps", name=f"psqk{j}_{sb}")
            for ci, (c0, cs) in enumerate(C_CHUNKS):
                nc.tensor.matmul(
                    pq[:rows, :W], wqk[ci][:cs, col0 : col0 + rows],
                    xnt[ci][:cs, :W],
                    start=(ci == 0), stop=(ci == NCC - 1),
                )
            t = qkt_pool.tile([128, W], FP16, tag=f"qkt{j}", name=f"qkt{j}_{sb}")
            if n_ % 2 == 0:
                nc.vector.tensor_scalar_add(
                    t[:rows, :], pq[:rows, :W], qkb[:rows, j : j + 1]
                )
            else:
                nc.scalar.activation(
                    t[:rows, :], pq[:rows, :W],
                    mybir.ActivationFunctionType.Identity, bias=qkb[:rows, j : j + 1],
                )
            qkt[j] = t
        # K-tail (rows 64:128 of group 4) shifted to partition 0
        t9 = qkt_pool.tile([64, W], FP16, tag="qkt9", name=f"qkt9_{sb}")
        nc.sync.dma_start(t9[0:64, :], qkt[4][64:128, :])
        qkt[9] = t9

        # ---- attention: software-pipelined head loop ----
        vgroups = vstaged.pop(sb, {})
        estate = {}
        b1state = {}
        onorm = [
            onpool.tile([128, W], FP16, tag=f"on{h}", name=f"on{h}_{sb}")
            for h in range(H)
        ]

        def emit_v_chunk(g, k, sb=sb, xnt=xnt, vgroups=vgroups):
            emit_v_chunk_into(sb, g, k, xnt, vgroups)

        def emit_v_chunk_into(sb, g, k, xnt, vgroups):
            n0, ns = V_CHUNKS[g]
            q, tj = k // 2, k % 2
            t0, ts_ = TOK_CHUNKS[tj]
            v = vpool.tile([128, 512], BF16, tag=f"v{q}{tj}",
                           name=f"v{q}{tj}g{g}_{sb}")
            pv = ps.tile([128, 512], FP32, tag="ps", name=f"psv{q}{tj}{g}_{sb}")
            for ci, (c0, cs) in enumerate(C_CHUNKS):
                nc.tensor.matmul(
                    pv[:ts_, :ns], xnt[ci][:cs, q * N + t0 : q * N + t0 + ts_],
                    wv[ci][:cs, n0 : n0 + ns],
                    start=(ci == 0), stop=(ci == NCC - 1),
                )
            if has_vbias:
                nc.vector.tensor_add(
                    v[:ts_, :ns], pv[:ts_, :ns], vb[:ts_, n0 : n0 + ns]
                )
            elif (q + tj) % 2 == 0:
                nc.scalar.copy(v[:ts_, :ns], pv[:ts_, :ns])
            else:
                nc.vector.tensor_copy(v[:ts_, :ns], pv[:ts_, :ns])
            vgroups.setdefault(g, {})[(q, tj)] = v

        def stage_a(h, sb=sb, qkt=qkt, estate=estate):
            # seeds + scores + exp for head h; S.T in [key, query] layout
            qg, kg, ro = _qk_slice(h)
            qt, kt = qkt[qg], qkt[kg]
            # e layout: [key-chunk, batch, query] so each key-chunk's
            # 392 columns are contiguous (dn needs a 1-D moving AP)
            e = epool.tile([128, 2 * W], BF16, tag="e", name=f"e{h}_{sb}")
            e4 = e[:, :].rearrange("p (c q n) -> p c q n", c=2, q=2)
            for q in range(SB):
                s = ps.tile([128, 512], FP32, tag="ps", name=f"st{q}h{h}_{sb}")
                for tj, (t0, ts_) in enumerate(TOK_CHUNKS):
                    nc.tensor.matmul(
                        s[:128, tj * N : tj * N + N],
                        ident[:128, :],
                        biasT[:, (h * 2 + tj) * N : (h * 2 + tj) * N + N],
                        start=True, stop=False, skip_group_check=True,
                    )
                    nc.tensor.matmul(
                        s[:128, tj * N : tj * N + N],
                        kt[ro : ro + 32, q * N + t0 : q * N + t0 + ts_],
                        qt[ro : ro + 32, q * N : q * N + N],
                        start=False, stop=True, skip_group_check=True,
                        tile_position=(ro, 0),
                    )
                st2 = s[:, 0 : 2 * N].rearrange("p (c n) -> p c n", c=2)
                nc.scalar.activation(e4[:, :, q], st2,
                                     mybir.ActivationFunctionType.Exp)
            estate[h] = e

        def stage_b1(h, sb=sb, estate=estate, b1state=b1state):
            # denominator, already broadcast: an all-ones stationary gives
            # every output partition the key-sum in one accumulated matmul
            e = estate[h]
            bcp = ps.tile([128, W], FP32, tag="ps", name=f"bcp{h}_{sb}")
            for tj in range(2):
                nc.tensor.matmul(
                    bcp[:, :W], allones[:, :], e[:, tj * W : tj * W + W],
                    start=(tj == 0), stop=(tj == 1),
                )
            rb = rbpool.tile([128, W], FP32, tag="rb", name=f"rb{h}_{sb}")
            nc.vector.reciprocal(rb[:, :], bcp[:, :W])
            b1state[h] = rb

        def stage_pv(h, sb=sb, estate=estate, vgroups=vgroups):
            e = estate.pop(h)
            g = h // 4
            vt = vgroups[g]
            n0, ns = V_CHUNKS[g]
            hcol = h * DV - n0
            ou = ps.tile([128, 512], FP32, tag="ps", name=f"ou{h}_{sb}")
            for q in range(SB):
                for tj, (t0, ts_) in enumerate(TOK_CHUNKS):
                    nc.tensor.matmul(
                        ou[:DV, q * N : q * N + N],
                        vt[(q, tj)][:ts_, hcol : hcol + DV],
                        e[:, tj * W + q * N : tj * W + q * N + N],
                        start=(tj == 0), stop=(tj == 1),
                    )
            return ou

        def stage_b3(h, ou, sb=sb, b1state=b1state, onorm=onorm):
            # normalize O.T into its resident fp16 slot (DVE)
            rb = b1state.pop(h)
            nc.vector.tensor_mul(
                onorm[h][:, :].rearrange("p (b n) -> p b n", b=2, n=N),
                ou[:DV, 0 : 2 * N].rearrange("p (b n) -> p b n", b=2, n=N),
                rb[:, :].rearrange("p (b n) -> p b n", b=2, n=N),
            )

        # pipeline: dn/recip lead by 1 head, broadcast trails so the PE
        # queue never waits on the DVE reciprocal
        for k in range(4):
            if (sb, 0, k) not in vstaged_done:
                emit_v_chunk(0, k)
        stage_a(0)
        stage_a(1)
        stage_b1(0)
        for h in range(H):
            g_next = h // 4 + 1
            if g_next <= 4 and (sb, g_next, h % 4) not in vstaged_done:
                emit_v_chunk(g_next, h % 4)
            if h + 2 < H:
                stage_a(h + 2)
            ou = stage_pv(h)
            if h + 1 < H:
                stage_b1(h + 1)
            stage_b3(h, ou)
            if pending_finals and h in (0, 2, 4, 6):
                pending_finals.pop(0)()
            if sb + 1 < NSB:
                if h in (5, 7, 9, 11):
                    k = (5, 7, 9, 11).index(h)
                    ln_stats(sb + 1, k // 2, k % 2)
                elif h in (6, 8, 10, 12):
                    k = (6, 8, 10, 12).index(h)
                    ln_apply(sb + 1, k // 2, k % 2)

        # ---- proj: accumulate heads from resident O.T, with the next
        # superbatch's xn transposes interleaved (PE stays busy on proj
        # while the transpose copies drain on ACT/DVE) ----
        ft = ftpool.tile([128, NCC * W], FP16, tag="ft", name=f"ft{sb}")
        if sb + 1 < NSB:
            xnt_next = [
                xnt_pool.tile([128, W], FP16, tag=f"xnt{ci}", name=f"xnt{ci}_{sb + 1}")
                for ci in range(NCC)
            ]
            xnt_staged[sb + 1] = xnt_next
        # m-outer / h-inner: one PSUM bank at a time (onorm re-reads from
        # SBUF are free), leaving banks for the interleaved transposes
        if sb + 1 < NSB:
            vg_next = {}
            vstaged[sb + 1] = vg_next
        for m in range(NCC):
            c0, mc = C_CHUNKS[m]
            if sb + 1 < NSB and m < 4:
                emit_transpose(sb + 1, m // 2, m % 2, xnt_next)
            if sb + 1 < NSB and m in (3, 4):
                k = m - 3
                emit_v_chunk_into(sb + 1, 0, k, xnt_next, vg_next)
                vstaged_done.add((sb + 1, 0, k))
            pp = ps.tile([128, W], FP32, tag="ps", name=f"pp{m}_{sb}")
            for h in range(H):
                nc.tensor.matmul(
                    pp[:mc, :W], pw[:, h * C + c0 : h * C + c0 + mc],
                    onorm[h][:DV, :W],
                    start=(h == 0), stop=(h == H - 1),
                )
            nc.vector.tensor_scalar_add(
                ft[:mc, m * W : m * W + W], pp[:mc, :W], pb[:mc, m : m + 1]
            )

        # ---- transpose back to token layout and store (deferred into the
        # next superbatch's head loop; flushed immediately on the last) ----
        def make_final(q, tj, ft=ft, sb=sb):
            def emit():
                b = sb * SB + q
                t0, ts_ = TOK_CHUNKS[tj]
                f = fpool.tile([128, C], FP32, tag="f", name=f"f{q}{tj}_{sb}")
                for m, (c0, mc) in enumerate(C_CHUNKS):
                    pt = ps.tile([128, 512], FP16, tag="ps", name=f"psf{m}{q}{tj}_{sb}")
                    src0 = m * W + q * N + t0
                    nc.tensor.transpose(
                        pt[:ts_, :mc], ft[:mc, src0 : src0 + ts_], ident[:mc, :mc]
                    )
                    if m % 2 == 0:
                        nc.scalar.copy(f[:ts_, c0 : c0 + mc], pt[:ts_, :mc])
                    else:
                        nc.vector.tensor_copy(f[:ts_, c0 : c0 + mc], pt[:ts_, :mc])
                nc.sync.dma_start(out_d[b, t0 : t0 + ts_, :], f[:ts_, :])
            return emit

        chunks = [make_final(q, tj) for q in range(SB) for tj in range(2)]
        if sb + 1 < NSB:
            pending_finals.extend(chunks)
        else:
            for c in chunks:
                c()


def _build(has_vbias: bool):
    nc = bass.Bass(
        "TRN2", target_bir_lowering=False, debug=False,
        enable_asserts=False, num_devices=NCORES,
    )
    aps = {}
    aps["x"] = nc.dram_tensor("x", [BPC, N, C], FP32, kind="ExternalInput").ap()
    aps["wqkt"] = nc.dram_tensor("wqkt", [C, 2 * C], FP16, kind="ExternalInput").ap()
    aps["wvt"] = nc.dram_tensor("wvt", [C, DH], FP16, kind="ExternalInput").ap()
    aps["pwp"] = nc.dram_tensor("pwp", [128, H * C], FP16, kind="ExternalInput").ap()
    aps["biasp"] = nc.dram_tensor("biasp", [128, H * W], FP16, kind="ExternalInput").ap()
    aps["qkb"] = nc.dram_tensor("qkb", [128, 10], FP32, kind="ExternalInput").ap()
    aps["pb"] = nc.dram_tensor("pb", [128, NCC], FP32, kind="ExternalInput").ap()
    if has_vbias:
        aps["vb"] = nc.dram_tensor("vb", [128, DH], FP32, kind="ExternalInput").ap()
    aps["out"] = nc.dram_tensor("out", [BPC, N, C], FP32, kind="ExternalOutput").ap()

    with tile.TileContext(nc) as tc, ExitStack() as ctx:
        with nc.allow_low_precision(reason="fp16/bf16 matmul pipeline"):
            _emit(ctx, tc, aps, has_vbias)
    _split_multiwaits(nc)
    return nc


_BUILD_CACHE: dict = {}


def _prep_host(x, ln_g, ln_b, qkv_w, qkv_b, proj_w, proj_b, attn_biases, bias_idxs):
    """Permute/fold weights host-side. Returns (in_map_consts, has_vbias)."""
    f32 = np.float32
    f16 = np.float16
    qkv_w = np.asarray(qkv_w, f32)
    qkv_b = np.asarray(qkv_b, f32)
    ln_g = np.asarray(ln_g, f32)
    ln_b = np.asarray(ln_b, f32)
    proj_w = np.asarray(proj_w, f32)
    proj_b = np.asarray(proj_b, f32)
    attn_biases = np.asarray(attn_biases, f32)
    bias_idxs = np.asarray(bias_idxs)

    per = 2 * KD + DV  # 192 rows per head in qkv_w
    wq = np.concatenate([qkv_w[h * per : h * per + KD] for h in range(H)], 0)
    wk = np.concatenate([qkv_w[h * per + KD : h * per + 2 * KD] for h in range(H)], 0)
    wv = np.concatenate([qkv_w[h * per + 2 * KD : (h + 1) * per] for h in range(H)], 0)
    bq = np.concatenate([qkv_b[h * per : h * per + KD] for h in range(H)], 0)
    bk = np.concatenate([qkv_b[h * per + KD : h * per + 2 * KD] for h in range(H)], 0)
    bv = np.concatenate([qkv_b[h * per + 2 * KD : (h + 1) * per] for h in range(H)], 0)

    # fold LN affine: xn = xn0 * g + beta  =>  W_eff = W*g, b_eff = W@beta + b
    wq_eff = (wq * ln_g[None, :] * SCALE).astype(f32)
    wk_eff = (wk * ln_g[None, :]).astype(f32)
    wv_eff = (wv * ln_g[None, :]).astype(f32)
    bq_eff = ((wq @ ln_b + bq) * SCALE).astype(f32)
    bk_eff = (wk @ ln_b + bk).astype(f32)
    bv_eff = (wv @ ln_b + bv).astype(f32)

    # group-contiguous packing: [Q 0:512 | Q 512:576 | K 512:576 | K 0:512]
    wqkt = np.concatenate(
        [wq_eff.T[:, 0:512], wq_eff.T[:, 512:576],
         wk_eff.T[:, 512:576], wk_eff.T[:, 0:512]], axis=1
    ).astype(f16).copy()
    wvt = wv_eff.T.astype(f16).copy()
    # proj_w.T packed per head: [128 (dv), 18*576]
    pwp = np.ascontiguousarray(
        proj_w.T.reshape(H, DV, C).transpose(1, 0, 2).reshape(DV, H * C)
    ).astype(f16)

    # packed qk bias: 9 groups [4x128 q, (q-tail;k-tail), 4x128 k]
    qkb = np.zeros((128, 10), f32)
    for j in range(4):
        qkb[:, j] = bq_eff[128 * j : 128 * j + 128]
        qkb[:, 5 + j] = bk_eff[128 * j : 128 * j + 128]
    qkb[0:64, 4] = bq_eff[512:576]
    qkb[64:128, 4] = bk_eff[512:576]
    pb = np.zeros((128, NCC), f32)
    for m, (c0, mc) in enumerate(C_CHUNKS):
        pb[:mc, m] = proj_b[c0 : c0 + mc]

    # seed bias, [key, query] per head, overlapping key chunks (0,128) and
    # (68,128); the second chunk's first 60 rows (keys 68:128, already
    # covered by chunk 0) get -60000 so exp gives exact zeros.
    biasT = attn_biases[:, bias_idxs.T]                 # [H, N(key), N(query)]
    biasp = np.zeros((128, H * W), f32)
    for h in range(H):
        biasp[:, h * W : h * W + N] = biasT[h, 0:128, :]
        biasp[0:60, h * W + N : h * W + 2 * N] = -60000.0
        biasp[60:128, h * W + N : h * W + 2 * N] = biasT[h, 128:196, :]
    biasp = biasp.astype(f16)

    has_vbias = bool(np.any(bv_eff != 0.0))
    consts = {
        "wqkt": wqkt, "wvt": wvt, "pwp": pwp,
        "biasp": np.ascontiguousarray(biasp),
        "qkb": qkb, "pb": pb,
    }
    if has_vbias:
        consts["vb"] = np.broadcast_to(bv_eff[None, :], (128, DH)).copy()
    return consts, has_vbias


def kernel(**inputs) -> np.ndarray:
    x = np.asarray(inputs["x"], np.float32)
    consts, has_vbias = _prep_host(
        x, inputs["ln_g"], inputs["ln_b"], inputs["qkv_w"], inputs["qkv_b"],
        inputs["proj_w"], inputs["proj_b"], inputs["attn_biases"],
        inputs["bias_idxs"],
    )
    key = has_vbias
    if key not in _BUILD_CACHE:
        _BUILD_CACHE[key] = _build(has_vbias)
    nc = _BUILD_CACHE[key]

    in_maps = []
    for c in range(NCORES):
        m = {"x": np.ascontiguousarray(x[c * BPC : (c + 1) * BPC])}
        m.update(consts)
        in_maps.append(m)
    res = run_bass_kernel_spmd(nc, in_maps, list(range(NCORES)))
    out = np.concatenate([res.results[c]["out"] for c in range(NCORES)], axis=0)
    return out.astype(np.float32)


# revision 48
# speedup vs baseline: 1.0948x; 1.0066x over previous
"""Trainium2 Bass kernel for the LeViT-style attention block.

Contract: kernel(**inputs) takes the FULL unsharded inputs (numpy) and
returns the FULL [128, 196, 576] float32 output. Internally shards the
batch dim across 8 NeuronCores (16 batches per core) and runs a single
SPMD Bass/Tile program via run_bass_kernel_spmd.

Math (per batch b):
  xn   = LayerNorm(x[b]) * g + beta                     [196, 576]
  qkv  = xn @ qkv_w.T + qkv_b      -> q,k,v per head
  S_h  = (q_h * kd^-0.5) @ k_h.T + bias_h               [196, 196]
  P_h  = softmax(S_h, axis=-1)
  O_h  = P_h @ v_h                                      [196, 128]
  out  = concat_h(O_h) @ proj_w.T + proj_b              [196, 576]

v3 implementation notes (cost-model driven):
  - matmul cost = out-free-size x cycles/row of the MOVING operand
    (fp16/bf16 = 1 always; f32r = 1 only when free >= 256; fp32 = 4);
    contraction depth, partition fill and stationary loads are free, so
    everything 2-byte-able is fp16 (precision) or bf16 (range).
  - token chunks are the OVERLAPPING pairs (0,128) and (68,128): every
    PE output keeps a full, base-0 partition range.  The 60-key overlap
    in the second chunk is killed with -60000 rows in the bias seed
    (exp -> exact 0), so denominator/PV stay correct.
  - scores: per-head bias seeded into PSUM by an identity matmul (fp16,
    196/row), score matmul accumulates on top; exp on ACT; denominator
    via ones-column matmul on PE (PSUM-accumulated across key chunks);
    reciprocal on DVE; ones-row broadcast matmul on PE; normalize fused
    into the PSUM->SBUF O.T copy on DVE.
  - per-head O.T resident in SBUF fp16 (no DRAM scratch); proj streams
    it with fp16 moving operands into 5 resident PSUM banks.
  - weights/bias constants ship as fp16 in a handful of large DMAs on
    the SP (HWDGE) queue, after the first x loads, ordered by first use
    (the Pool/SWDGE queue's per-DMA descriptor-gen cost would starve
    the first superbatches).
  - PSUM->SBUF copies are spread over Pool/ACT/DVE to keep every
    engine under the PE budget.
"""

import os

os.environ.setdefault("MYCRO_LOCAL_CACHE", "1")

from contextlib import ExitStack

import numpy as np
import ml_dtypes

import concourse.bass as bass
import concourse.mybir as mybir
import concourse.tile as tile
from concourse import masks
from concourse.bass import AP
from concourse.bass_utils import run_bass_kernel_spmd

# Problem shape (hardcoded per contest contract).
B, N, C = 128, 196, 576
H, KD, DV = 18, 32, 128
DH = H * DV            # 2304
LN_EPS = 1e-5
SCALE = KD ** -0.5
NCORES = 8
BPC = B // NCORES      # 16 batches per core
SB = 2                 # batches per "superbatch" iteration
NSB = BPC // SB        # 8
W = SB * N             # 392: packed two-batch free dim

FP32 = mybir.dt.float32
F32R = mybir.dt.float32r
FP16 = mybir.dt.float16
BF16 = mybir.dt.bfloat16

# token-dim chunks: overlapping full-128 chunks
TOK_CHUNKS = [(0, 128), (68, 128)]
# C-dim chunks (576 = 4*128 + 64)
C_CHUNKS = [(i * 128, min(128, C - i * 128)) for i in range((C + 127) // 128)]
NCC = len(C_CHUNKS)
# V free-dim chunks of 512 = 4 heads
V_CHUNKS = [(i * 512, min(512, DH - i * 512)) for i in range((DH + 511) // 512)]


def _split_multiwaits(nc):
    """This container's walrus rejects >1 sync-wait per instruction
    (TPB EVENTS struct has a single wait slot). Split extras into
    preceding same-engine NOPs — semantically identical."""
    for f in nc.m.functions:
        for blk in f.blocks:
            newlist = []
            changed = False
            for inst in blk.instructions:
                si = inst.sync_info
                waits = list(si.on_wait) if si is not None else []
                if len(waits) > 1:
                    changed = True
                    for j, w in enumerate(waits[:-1]):
                        nop = mybir.InstNoOp(name=f"{inst.name}_sw{j}", ins=[], outs=[])
                        nop.engine = inst.engine
                        nop.sync_info = mybir.SyncInfo(on_wait=[w], on_update=[])
                        newlist.append(nop)
                    inst.sync_info = mybir.SyncInfo(
                        on_wait=[waits[-1]], on_update=list(si.on_update)
                    )
                newlist.append(inst)
            if changed:
                blk.instructions = newlist


# 10 QK output groups over the packed wqkt columns
# [Q 0:512 | Q 512:576 | K 512:576 | K 0:512]: 4x128 Q, 64 Q-tail,
# 64 K-tail, 4x128 K. Within every group a head's 32 rows sit at the
# same offset for q and k (the PE pairs moving/stationary rows by
# absolute partition, so offsets must match).
QK_GROUPS = [(128 * j, 128) for j in range(4)] + [(512, 128)] + [
    (640 + 128 * j, 128) for j in range(4)
]


def _qk_slice(h):
    """(qgroup, kgroup, offset) for head h. Group 4 holds [Q-tail;
    K-tail]; the K-tail is DMA-shifted to partition 0 in tile 9 so the
    per-head q/k offsets match."""
    if h < 16:
        return h // 4, 5 + h // 4, (32 * h) % 128
    return 4, 9, 32 * (h - 16)


def _emit(ctx: ExitStack, tc: tile.TileContext, aps: dict, has_vbias: bool):
    nc = tc.nc
    x_d = aps["x"]          # [BPC, 196, 576] f32
    out_d = aps["out"]      # [BPC, 196, 576] f32
    wqk_d = aps["wqkt"]     # [576, 1152] fp16, packed as
                            # [Wq.T rows0:512 | Wq.T 512:576 | Wk.T 512:576 |
                            #  Wk.T rows0:512] so all 9 groups are contiguous
    wv_d = aps["wvt"]       # [576, 2304] fp16 (Wv.T, head-major columns)
    pw_d = aps["pwp"]       # [128, 18*576] fp16 (proj_w.T packed per head)
    bias_d = aps["biasp"]   # [128, 18*392] fp16 (seed bias, chunked
                            #  [head, key-chunk, query]; overlap rows -60000)
    qkb_d = aps["qkb"]      # [128, 9] f32 (per-group qk bias columns)
    pb_d = aps["pb"]        # [128, 5] f32 (proj bias chunks)
    vb_d = aps.get("vb")    # [128, 2304] f32 (replicated v bias) — optional

    cpool = ctx.enter_context(tc.tile_pool(name="consts", bufs=1))
    xpool = ctx.enter_context(tc.tile_pool(name="x", bufs=2))
    xnpool = ctx.enter_context(tc.tile_pool(name="xn", bufs=2))
    stat = ctx.enter_context(tc.tile_pool(name="stat", bufs=2))
    sqpool = ctx.enter_context(tc.tile_pool(name="sq", bufs=2))
    xnt_pool = ctx.enter_context(tc.tile_pool(name="xnt", bufs=1))
    qkt_pool = ctx.enter_context(tc.tile_pool(name="qkt", bufs=1))
    vpool = ctx.enter_context(tc.tile_pool(name="v", bufs=2))
    epool = ctx.enter_context(tc.tile_pool(name="e", bufs=4))
    rbpool = ctx.enter_context(tc.tile_pool(name="rb", bufs=3))
    onpool = ctx.enter_context(tc.tile_pool(name="on", bufs=1))
    ftpool = ctx.enter_context(tc.tile_pool(name="ft", bufs=1))
    fpool = ctx.enter_context(tc.tile_pool(name="f", bufs=2))
    ps = ctx.enter_context(tc.tile_pool(name="ps", bufs=8, space="PSUM"))

    # ---- small on-chip constants ----
    ident = cpool.tile([128, 128], FP16, tag="ident", name="ident")
    masks.make_identity(nc, ident[:])
    allones = cpool.tile([128, 128], BF16, tag="allones", name="allones")
    nc.gpsimd.memset(allones[:], 1.0)
    epsb = cpool.tile([128, 1], FP32, tag="epsb", name="epsb")
    nc.gpsimd.memset(epsb[:], LN_EPS)

    # ---- resident weight tiles (DMAs issued after the first x loads so
    # the SP queue serves x first; ordered by first use) ----
    wqk = [cpool.tile([128, 2 * C], FP16, tag=f"wqk{ci}", name=f"wqk{ci}")
           for ci in range(NCC)]
    wv = [cpool.tile([128, DH], FP16, tag=f"wv{ci}", name=f"wv{ci}")
          for ci in range(NCC)]
    pw = cpool.tile([128, H * C], FP16, tag="pw", name="pw")
    biasT = cpool.tile([128, H * W], FP16, tag="biasT", name="biasT")
    qkb = cpool.tile([128, 10], FP32, tag="qkb", name="qkb")
    pb = cpool.tile([128, NCC], FP32, tag="pb", name="pb")
    vb = None
    if has_vbias:
        vb = cpool.tile([128, DH], FP32, tag="vb", name="vb")

    def load_weights():
        for ci, (c0, cs) in enumerate(C_CHUNKS):
            nc.sync.dma_start(wqk[ci][:cs, :], wqk_d[c0 : c0 + cs, :])
        nc.sync.dma_start(qkb[:], qkb_d[:])
        nc.sync.dma_start(biasT[:], bias_d[:])
        for ci, (c0, cs) in enumerate(C_CHUNKS):
            nc.sync.dma_start(wv[ci][:cs, :], wv_d[c0 : c0 + cs, :])
        if has_vbias:
            nc.sync.dma_start(vb[:], vb_d[:])
        nc.sync.dma_start(pw[:], pw_d[:])
        nc.sync.dma_start(pb[:], pb_d[:])

    inv_c = 1.0 / C

    lnstate = {}
    lnmid = {}
    xstate = {}

    def ln_load(sbx, q, tj):
        b = sbx * SB + q
        t0, ts_ = TOK_CHUNKS[tj]
        xt = xpool.tile([128, C], FP32, tag=f"xb{q}{tj}", name=f"xb{q}{tj}_{sbx}")
        nc.sync.dma_start(xt[:ts_, :], x_d[b, t0 : t0 + ts_, :])
        xstate[(sbx, q, tj)] = xt

    def ln_chunk(sbx, q, tj):
        ln_stats(sbx, q, tj)
        ln_apply(sbx, q, tj)

    def ln_stats(sbx, q, tj):
        """LayerNorm stats for one loaded (batch, token-chunk)."""
        t0, ts_ = TOK_CHUNKS[tj]
        xt = xstate.pop((sbx, q, tj))
        negmu = stat.tile([128, 1], FP32, tag="negmu", name=f"nm{q}{tj}_{sbx}")
        nc.vector.tensor_reduce(
            negmu[:ts_], xt[:ts_, :], axis=mybir.AxisListType.X,
            op=mybir.AluOpType.add, negate=True,
        )
        nc.vector.tensor_scalar_mul(negmu[:ts_], negmu[:ts_], inv_c)
        sq = sqpool.tile([128, C], FP32, tag="sq", name=f"sq{q}{tj}_{sbx}")
        ha = stat.tile([128, 1], FP32, tag="ha", name=f"ha{q}{tj}_{sbx}")
        nc.scalar.activation(
            sq[:ts_, :], xt[:ts_, :],
            mybir.ActivationFunctionType.Square, bias=negmu[:ts_], accum_out=ha[:ts_],
        )
        std = stat.tile([128, 1], FP32, tag="std", name=f"std{q}{tj}_{sbx}")
        nc.scalar.activation(
            std[:ts_], ha[:ts_], mybir.ActivationFunctionType.Sqrt,
            bias=epsb[:ts_], scale=inv_c,
        )
        r = stat.tile([128, 1], FP32, tag="r", name=f"r{q}{tj}_{sbx}")
        nc.vector.reciprocal(r[:ts_], std[:ts_])
        negmur = stat.tile([128, 1], FP32, tag="negmur", name=f"nr{q}{tj}_{sbx}")
        nc.vector.tensor_mul(negmur[:ts_], negmu[:ts_], r[:ts_])
        lnmid[(sbx, q, tj)] = (xt, negmur, r)

    def ln_apply(sbx, q, tj):
        """normalize in fp16 from the precomputed stats."""
        t0, ts_ = TOK_CHUNKS[tj]
        xt, negmur, r = lnmid.pop((sbx, q, tj))
        xn = xnpool.tile([128, C], FP16, tag=f"xn{q}{tj}", name=f"xn{q}{tj}_{sbx}")
        nc.vector.tensor_scalar(
            xn[:ts_, :], xt[:ts_, :], r[:ts_], negmur[:ts_],
            op0=mybir.AluOpType.mult, op1=mybir.AluOpType.add,
        )
        lnstate[(sbx, q, tj)] = xn

    pending_finals = []

    for q in range(SB):
        for tj in range(len(TOK_CHUNKS)):
            ln_load(0, q, tj)
    load_weights()
    for q in range(SB):
        for tj in range(len(TOK_CHUNKS)):
            ln_chunk(0, q, tj)

    xnt_staged = {}
    vstaged = {}
    vstaged_done = set()

    def emit_v_chunk_into(sbx, g, k, xnt, vgroups):
        n0, ns = V_CHUNKS[g]
        q, tj = k // 2, k % 2
        t0, ts_ = TOK_CHUNKS[tj]
        v = vpool.tile([128, 512], BF16, tag=f"v{q}{tj}",
                       name=f"v{q}{tj}g{g}_{sbx}")
        pv = ps.tile([128, 512], FP32, tag="ps", name=f"psv{q}{tj}{g}_{sbx}")
        for ci, (c0, cs) in enumerate(C_CHUNKS):
            nc.tensor.matmul(
                pv[:ts_, :ns], xnt[ci][:cs, q * N + t0 : q * N + t0 + ts_],
                wv[ci][:cs, n0 : n0 + ns],
                start=(ci == 0), stop=(ci == NCC - 1),
            )
        if has_vbias:
            nc.vector.tensor_add(
                v[:ts_, :ns], pv[:ts_, :ns], vb[:ts_, n0 : n0 + ns]
            )
        else:
            nc.vector.tensor_copy(v[:ts_, :ns], pv[:ts_, :ns])
        vgroups.setdefault(g, {})[(q, tj)] = v

    def emit_transpose(sbx, q, tj, xnt):
        t0, ts_ = TOK_CHUNKS[tj]
        xn = lnstate.pop((sbx, q, tj))
        for ci, (c0, cs) in enumerate(C_CHUNKS):
            pt = ps.tile([128, 512], FP16, tag="ps", name=f"pst{q}{tj}{ci}_{sbx}")
            nc.tensor.transpose(
                pt[:cs, :ts_], xn[:ts_, c0 : c0 + cs], ident[:ts_, :ts_]
            )
            col = q * N + t0
            nc.vector.tensor_copy(xnt[ci][:cs, col : col + ts_], pt[:cs, :ts_])

    for sb in range(NSB):
        # ---- xn -> xnT (channel-major): staged during the previous
        # superbatch's proj pass, except for the first superbatch ----
        if sb in xnt_staged:
            xnt = xnt_staged.pop(sb)
        else:
            xnt = [
                xnt_pool.tile([128, W], FP16, tag=f"xnt{ci}", name=f"xnt{ci}_{sb}")
                for ci in range(NCC)
            ]
            for q in range(SB):
                for tj in range(len(TOK_CHUNKS)):
                    emit_transpose(sb, q, tj, xnt)

        if sb + 1 < NSB:
            for q in range(SB):
                for tj in range(len(TOK_CHUNKS)):
                    ln_load(sb + 1, q, tj)

        # ---- Q.T / K.T GEMMs (9 groups of 128 rows, both batches) ----
        qkt = [None] * 10
        for n_, j in enumerate((0, 5, 1, 6, 2, 7, 3, 8, 4)):
            col0, rows = QK_GROUPS[j]
            pq = ps.tile([128, 512], FP32, tag="ps", name=f"psqk{j}_{sb}")
            for ci, (c0, cs) in enumerate(C_CHUNKS):
                nc.tensor.matmul(
                    pq[:rows, :W], wqk[ci][:cs, col0 : col0 + rows],
                    xnt[ci][:cs, :W],
                    start=(ci == 0), stop=(ci == NCC - 1),
                )
            t = qkt_pool.tile([128, W], FP16, tag=f"qkt{j}", name=f"qkt{j}_{sb}")
            if n_ % 2 == 0:
                nc.vector.tensor_scalar_add(
                    t[:rows, :], pq[:rows, :W], qkb[:rows, j : j + 1]
                )
            else:
                nc.scalar.activation(
                    t[:rows, :], pq[:rows, :W],
                    mybir.ActivationFunctionType.Identity, bias=qkb[:rows, j : j + 1],
                )
            qkt[j] = t
        # K-tail (rows 64:128 of group 4) shifted to partition 0
        t9 = qkt_pool.tile([64, W], FP16, tag="qkt9", name=f"qkt9_{sb}")
        nc.sync.dma_start(t9[0:64, :], qkt[4][64:128, :])
        qkt[9] = t9

        # ---- attention: software-pipelined head loop ----
        vgroups = vstaged.pop(sb, {})
        estate = {}
        b1state = {}
        onorm = [
            onpool.tile([128, W], FP16, tag=f"on{h}", name=f"on{h}_{sb}")
            for h in range(H)
        ]

        def emit_v_chunk(g, k, sb=sb, xnt=xnt, vgroups=vgroups):
            emit_v_chunk_into(sb, g, k, xnt, vgroups)

        def stage_a(h, sb=sb, qkt=qkt, estate=estate):
            # seeds + scores + exp for head h; S.T in [key, query] layout
            qg, kg, ro = _qk_slice(h)
            qt, kt = qkt[qg], qkt[kg]
            # e layout: [key-chunk, batch, query] so each key-chunk's
            # 392 columns are contiguous (dn needs a 1-D moving AP)
            e = epool.tile([128, 2 * W], BF16, tag="e", name=f"e{h}_{sb}")
            e4 = e[:, :].rearrange("p (c q n) -> p c q n", c=2, q=2)
            for q in range(SB):
                s = ps.tile([128, 512], FP32, tag="ps", name=f"st{q}h{h}_{sb}")
                for tj, (t0, ts_) in enumerate(TOK_CHUNKS):
                    nc.tensor.matmul(
                        s[:128, tj * N : tj * N + N],
                        ident[:128, :],
                        biasT[:, (h * 2 + tj) * N : (h * 2 + tj) * N + N],
                        start=True, stop=False, skip_group_check=True,
                    )
                    nc.tensor.matmul(
                        s[:128, tj * N : tj * N + N],
                        kt[ro : ro + 32, q * N + t0 : q * N + t0 + ts_],
                        qt[ro : ro + 32, q * N : q * N + N],
                        start=False, stop=True, skip_group_check=True,
                        tile_position=(ro, 0),
                    )
                st2 = s[:, 0 : 2 * N].rearrange("p (c n) -> p c n", c=2)
                nc.scalar.activation(e4[:, :, q], st2,
                                     mybir.ActivationFunctionType.Exp)
            estate[h] = e

        def stage_b1(h, sb=sb, estate=estate, b1state=b1state):
            # denominator, already broadcast: an all-ones stationary gives
            # every output partition the key-sum in one accumulated matmul
            e = estate[h]
            bcp = ps.tile([128, W], FP32, tag="ps", name=f"bcp{h}_{sb}")
            for tj in range(2):
                nc.tensor.matmul(
                    bcp[:, :W], allones[:, :], e[:, tj * W : tj * W + W],
                    start=(tj == 0), stop=(tj == 1),
                )
            rb = rbpool.tile([128, W], FP32, tag="rb", name=f"rb{h}_{sb}")
            nc.vector.reciprocal(rb[:, :], bcp[:, :W])
            b1state[h] = rb

        def stage_pv(h, sb=sb, estate=estate, vgroups=vgroups):
            e = estate.pop(h)
            g = h // 4
            vt = vgroups[g]
            n0, ns = V_CHUNKS[g]
            hcol = h * DV - n0
            ou = ps.tile([128, 512], FP32, tag="ps", name=f"ou{h}_{sb}")
            for q in range(SB):
                for tj, (t0, ts_) in enumerate(TOK_CHUNKS):
                    nc.tensor.matmul(
                        ou[:DV, q * N : q * N + N],
                        vt[(q, tj)][:ts_, hcol : hcol + DV],
                        e[:, tj * W + q * N : tj * W + q * N + N],
                        start=(tj == 0), stop=(tj == 1),
                    )
            return ou

        def stage_b3(h, ou, sb=sb, b1state=b1state, onorm=onorm):
            # normalize O.T into its resident fp16 slot (DVE)
            rb = b1state.pop(h)
            nc.vector.tensor_mul(
                onorm[h][:, :].rearrange("p (b n) -> p b n", b=2, n=N),
                ou[:DV, 0 : 2 * N].rearrange("p (b n) -> p b n", b=2, n=N),
                rb[:, :].rearrange("p (b n) -> p b n", b=2, n=N),
            )

        # pipeline: dn/recip lead by 1 head, broadcast trails so the PE
        # queue never waits on the DVE reciprocal
        for k in range(4):
            if (sb, 0, k) not in vstaged_done:
                emit_v_chunk(0, k)
        stage_a(0)
        stage_a(1)
        stage_b1(0)
        for h in range(H):
            g_next = h // 4 + 1
            if g_next <= 4 and (sb, g_next, h % 4) not in vstaged_done:
                emit_v_chunk(g_next, h % 4)
            if h + 2 < H:
                stage_a(h + 2)
            ou = stage_pv(h)
            if h + 1 < H:
                stage_b1(h + 1)
            stage_b3(h, ou)
            if pending_finals and h in (1, 3, 5, 7):
                pending_finals.pop(0)()
            if sb + 1 < NSB:
                if h in (5, 7, 9, 11):
                    k = (5, 7, 9, 11).index(h)
                    ln_stats(sb + 1, k // 2, k % 2)
                elif h in (6, 8, 10, 12):
                    k = (6, 8, 10, 12).index(h)
                    ln_apply(sb + 1, k // 2, k % 2)

        # ---- proj: accumulate heads from resident O.T, with the next
        # superbatch's xn transposes interleaved (PE stays busy on proj
        # while the transpose copies drain on ACT/DVE) ----
        ft = ftpool.tile([128, NCC * W], FP16, tag="ft", name=f"ft{sb}")
        if sb + 1 < NSB:
            xnt_next = [
                xnt_pool.tile([128, W], FP16, tag=f"xnt{ci}", name=f"xnt{ci}_{sb + 1}")
                for ci in range(NCC)
            ]
            xnt_staged[sb + 1] = xnt_next
        # m-outer / h-inner: one PSUM bank at a time (onorm re-reads from
        # SBUF are free), leaving banks for the interleaved transposes
        if sb + 1 < NSB:
            vg_next = {}
            vstaged[sb + 1] = vg_next
        if sb + 1 == NSB:
            flast = {
                (q, tj): fpool.tile([128, C], FP32, tag=f"fl{q}{tj}",
                                    name=f"fl{q}{tj}_{sb}")
                for q in range(SB) for tj in range(2)
            }
        for m in range(NCC):
            c0, mc = C_CHUNKS[m]
            if sb + 1 < NSB and m < 4:
                emit_transpose(sb + 1, m // 2, m % 2, xnt_next)
            if sb + 1 < NSB and m in (3, 4):
                k = m - 3
                emit_v_chunk_into(sb + 1, 0, k, xnt_next, vg_next)
                vstaged_done.add((sb + 1, 0, k))
            pp = ps.tile([128, W], FP32, tag="ps", name=f"pp{m}_{sb}")
            for h in range(H):
                nc.tensor.matmul(
                    pp[:mc, :W], pw[:, h * C + c0 : h * C + c0 + mc],
                    onorm[h][:DV, :W],
                    start=(h == 0), stop=(h == H - 1),
                )
            nc.vector.tensor_scalar_add(
                ft[:mc, m * W : m * W + W], pp[:mc, :W], pb[:mc, m : m + 1]
            )
            if sb + 1 == NSB:
                # flush the last superbatch's outputs per m-chunk so the
                # final transposes/copies pipeline behind proj
                for q in range(SB):
                    for tj, (t0, ts_) in enumerate(TOK_CHUNKS):
                        pt = ps.tile([128, 512], FP16, tag="ps",
                                     name=f"psl{m}{q}{tj}_{sb}")
                        src0 = m * W + q * N + t0
                        nc.tensor.transpose(
                            pt[:ts_, :mc], ft[:mc, src0 : src0 + ts_],
                            ident[:mc, :mc],
                        )
                        f = flast[(q, tj)]
                        if (q + tj) % 2 == 0:
                            nc.scalar.copy(f[:ts_, c0 : c0 + mc], pt[:ts_, :mc])
                        else:
                            nc.vector.tensor_copy(f[:ts_, c0 : c0 + mc], pt[:ts_, :mc])

        # ---- transpose back to token layout and store (deferred into the
        # next superbatch's head loop; flushed immediately on the last) ----
        def make_final(q, tj, ft=ft, sb=sb):
            def emit():
                b = sb * SB + q
                t0, ts_ = TOK_CHUNKS[tj]
                f = fpool.tile([128, C], FP32, tag="f", name=f"f{q}{tj}_{sb}")
                for m, (c0, mc) in enumerate(C_CHUNKS):
                    pt = ps.tile([128, 512], FP16, tag="ps", name=f"psf{m}{q}{tj}_{sb}")
                    src0 = m * W + q * N + t0
                    nc.tensor.transpose(
                        pt[:ts_, :mc], ft[:mc, src0 : src0 + ts_], ident[:mc, :mc]
                    )
                    if m % 2 == 1:
                        nc.scalar.copy(f[:ts_, c0 : c0 + mc], pt[:ts_, :mc])
                    else:
                        nc.vector.tensor_copy(f[:ts_, c0 : c0 + mc], pt[:ts_, :mc])
                nc.sync.dma_start(out_d[b, t0 : t0 + ts_, :], f[:ts_, :])
            return emit

        if sb + 1 < NSB:
            pending_finals.extend(
                make_final(q, tj) for q in range(SB) for tj in range(2)
            )
        else:
            for q in range(SB):
                for tj, (t0, ts_) in enumerate(TOK_CHUNKS):
                    b = sb * SB + q
                    nc.sync.dma_start(
                        out_d[b, t0 : t0 + ts_, :], flast[(q, tj)][:ts_, :]
                    )


def _build(has_vbias: bool):
    nc = bass.Bass(
        "TRN2", target_bir_lowering=False, debug=False,
        enable_asserts=False, num_devices=NCORES,
    )
    aps = {}
    aps["x"] = nc.dram_tensor("x", [BPC, N, C], FP32, kind="ExternalInput").ap()
    aps["wqkt"] = nc.dram_tensor("wqkt", [C, 2 * C], FP16, kind="ExternalInput").ap()
    aps["wvt"] = nc.dram_tensor("wvt", [C, DH], FP16, kind="ExternalInput").ap()
    aps["pwp"] = nc.dram_tensor("pwp", [128, H * C], FP16, kind="ExternalInput").ap()
    aps["biasp"] = nc.dram_tensor("biasp", [128, H * W], FP16, kind="ExternalInput").ap()
    aps["qkb"] = nc.dram_tensor("qkb", [128, 10], FP32, kind="ExternalInput").ap()
    aps["pb"] = nc.dram_tensor("pb", [128, NCC], FP32, kind="ExternalInput").ap()
    if has_vbias:
        aps["vb"] = nc.dram_tensor("vb", [128, DH], FP32, kind="ExternalInput").ap()
    aps["out"] = nc.dram_tensor("out", [BPC, N, C], FP32, kind="ExternalOutput").ap()

    with tile.TileContext(nc) as tc, ExitStack() as ctx:
        with nc.allow_low_precision(reason="fp16/bf16 matmul pipeline"):
            _emit(ctx, tc, aps, has_vbias)
    _split_multiwaits(nc)
    return nc


_BUILD_CACHE: dict = {}


def _prep_host(x, ln_g, ln_b, qkv_w, qkv_b, proj_w, proj_b, attn_biases, bias_idxs):
    """Permute/fold weights host-side. Returns (in_map_consts, has_vbias)."""
    f32 = np.float32
    f16 = np.float16
    qkv_w = np.asarray(qkv_w, f32)
    qkv_b = np.asarray(qkv_b, f32)
    ln_g = np.asarray(ln_g, f32)
    ln_b = np.asarray(ln_b, f32)
    proj_w = np.asarray(proj_w, f32)
    proj_b = np.asarray(proj_b, f32)
    attn_biases = np.asarray(attn_biases, f32)
    bias_idxs = np.asarray(bias_idxs)

    per = 2 * KD + DV  # 192 rows per head in qkv_w
    wq = np.concatenate([qkv_w[h * per : h * per + KD] for h in range(H)], 0)
    wk = np.concatenate([qkv_w[h * per + KD : h * per + 2 * KD] for h in range(H)], 0)
    wv = np.concatenate([qkv_w[h * per + 2 * KD : (h + 1) * per] for h in range(H)], 0)
    bq = np.concatenate([qkv_b[h * per : h * per + KD] for h in range(H)], 0)
    bk = np.concatenate([qkv_b[h * per + KD : h * per + 2 * KD] for h in range(H)], 0)
    bv = np.concatenate([qkv_b[h * per + 2 * KD : (h + 1) * per] for h in range(H)], 0)

    # fold LN affine: xn = xn0 * g + beta  =>  W_eff = W*g, b_eff = W@beta + b
    wq_eff = (wq * ln_g[None, :] * SCALE).astype(f32)
    wk_eff = (wk * ln_g[None, :]).astype(f32)
    wv_eff = (wv * ln_g[None, :]).astype(f32)
    bq_eff = ((wq @ ln_b + bq) * SCALE).astype(f32)
    bk_eff = (wk @ ln_b + bk).astype(f32)
    bv_eff = (wv @ ln_b + bv).astype(f32)

    # group-contiguous packing: [Q 0:512 | Q 512:576 | K 512:576 | K 0:512]
    wqkt = np.concatenate(
        [wq_eff.T[:, 0:512], wq_eff.T[:, 512:576],
         wk_eff.T[:, 512:576], wk_eff.T[:, 0:512]], axis=1
    ).astype(f16).copy()
    wvt = wv_eff.T.astype(f16).copy()
    # proj_w.T packed per head: [128 (dv), 18*576]
    pwp = np.ascontiguousarray(
        proj_w.T.reshape(H, DV, C).transpose(1, 0, 2).reshape(DV, H * C)
    ).astype(f16)

    # packed qk bias: 9 groups [4x128 q, (q-tail;k-tail), 4x128 k]
    qkb = np.zeros((128, 10), f32)
    for j in range(4):
        qkb[:, j] = bq_eff[128 * j : 128 * j + 128]
        qkb[:, 5 + j] = bk_eff[128 * j : 128 * j + 128]
    qkb[0:64, 4] = bq_eff[512:576]
    qkb[64:128, 4] = bk_eff[512:576]
    pb = np.zeros((128, NCC), f32)
    for m, (c0, mc) in enumerate(C_CHUNKS):
        pb[:mc, m] = proj_b[c0 : c0 + mc]

    # seed bias, [key, query] per head, overlapping key chunks (0,128) and
    # (68,128); the second chunk's first 60 rows (keys 68:128, already
    # covered by chunk 0) get -60000 so exp gives exact zeros.
    biasT = attn_biases[:, bias_idxs.T]                 # [H, N(key), N(query)]
    biasp = np.zeros((128, H * W), f32)
    for h in range(H):
        biasp[:, h * W : h * W + N] = biasT[h, 0:128, :]
        biasp[0:60, h * W + N : h * W + 2 * N] = -60000.0
        biasp[60:128, h * W + N : h * W + 2 * N] = biasT[h, 128:196, :]
    biasp = biasp.astype(f16)

    has_vbias = bool(np.any(bv_eff != 0.0))
    consts = {
        "wqkt": wqkt, "wvt": wvt, "pwp": pwp,
        "biasp": np.ascontiguousarray(biasp),
        "qkb": qkb, "pb": pb,
    }
    if has_vbias:
        consts["vb"] = np.broadcast_to(bv_eff[None, :], (128, DH)).copy()
    return consts, has_vbias


def kernel(**inputs) -> np.ndarray:
    x = np.asarray(inputs["x"], np.float32)
    consts, has_vbias = _prep_host(
        x, inputs["ln_g"], inputs["ln_b"], inputs["qkv_w"], inputs["qkv_b"],
        inputs["proj_w"], inputs["proj_b"], inputs["attn_biases"],
        inputs["bias_idxs"],
    )
    key = has_vbias
    if key not in _BUILD_CACHE:
        _BUILD_CACHE[key] = _build(has_vbias)
    nc = _BUILD_CACHE[key]

    in_maps = []
    for c in range(NCORES):
        m = {"x": np.ascontiguousarray(x[c * BPC : (c + 1) * BPC])}
        m.update(consts)
        in_maps.append(m)
    res = run_bass_kernel_spmd(nc, in_maps, list(range(NCORES)))
    out = np.concatenate([res.results[c]["out"] for c in range(NCORES)], axis=0)
    return out.astype(np.float32)
